# revision 1
# baseline (speedup 1.0000x reference)
import base64, io, zlib
import numpy as np

# nn_ContractProduct3j: out[n,c,s] = sum_ij W[i,j,s] t1[n,c,i] t2[n,c,j]
# W (16,16,25) is the fused even-parity Wigner-3j tensor; it is symmetric in
# (i,j), so it admits a symmetric rank-64 (Waring) decomposition
#   W[i,j,s] = sum_r C[s,r] V[i,r] V[j,r]          (rel. Frobenius err 2.75e-3)
# giving out = ((t1+t2)/2 @ V)^2 - ((t1-t2)/2 @ V)^2, then @ C.T.
# Sharding: nodes split across the 8 NeuronCores (pure data parallel).
_WBLOB = "eNqlenk4V1/Ur6HIkDFKFBKKREXle85eSuYGQlQqZSoNhAY0GCJD5ojMc6YiKvl+z16KSpIGVEr9GjTTpEHRcPvd973De/+796797LOfvZ6zzvOcZ6+19lqftRyWio4xEPoP0hT65rW25M9/0lihCUIus/12hggLyQmFy/3HK/9jPbpi1XKHNcJCe4T26Xp5B3kG6vI0dFkfY91ZGro+/oG7Ajf5ufsHenn/y7fatD3I+y8/aMumnd5/9zPmzJ+lMd945iyNAxr/jyT5bmItDi8PwXXj20BlnCfkckHo89gXTfg+2L7DFxPTdsDIvQO4ujcIJlwLRr7CKfioEwwq9v4on3obizzvQahiENL5vujYuhWdP9lgV/EuHBDZh8t2lUNZnz9om+yDcMMQ7Kqwx6/ye8EtYCt8F/LF5o5ALPfYA/uVK/FbzF785lyIldd8cezqcxgxuhdUirah7WVf9LkcjLM89uKGRVdJVXEoBO7ege5yAbBSMRCznu5ARd4OrPTzwSMiahjxKRgDlpXh/Ru7wOpbDWiPVcZ/lBRhe1AKGg2E4dxb1fBiYjC6yvjh21mBWGxah+PnrMRtU/Oot9gjkjA0htwbZ4p5Z+YitbjILi89iT9X7YNS7itZfU8UjqeGE1TxxgOxMTgY6IY+MXxUCUqDiZKnaYFbLh1VGAsRCXn42jkeNsQkoL/PQ1YoXBw/VkphyTxV8uytDI5EjCMzwr5x3rl+bEbQYkxct4euNxaQ13QBZ2pkh2pPnpD+M+dwktgBdDOLA2WvG1zZtsf02KRx3Aw3gqyzAYqfLgUSnwINPSfJYlEhuiaqi75aFc4aXWnC35Ga0Cq+FXTVbGH9cidYzSyh1nvT8H5dGRicnEXqYivQmy8E8pEpaLpqIkrFzUPl6Rsh+U04Ks71RNywBro6K1nt65dA4KRLrsWFg8zWBgw5dgpEp1TAftSEAX8v+LVYEqqHlcn4oy5wzT8Xty2biVl+FXh601LytnMPaXxRRmVTlHD52RPAWOxD6ZMpsOuvntTJDBCBxjGauyEBi5SnAG51hAnK9vDs6AOiirepdk8etbI/TiMSPOGzWCJZ08XhonBKWvOTcUJkLlx4HIOlIj/Ii9rVeLFhJfssuQAWmo3D2LxbIJl7gFwrbIGaphT4vnYfST5XToXvD1Mr4w3wpOU9NTNIxbHav+m71fmsTHA4ebtMnBbbRKC6XB62bXPHWz7JaKWzDvc8yMQJURowOr2LuI+XAN8PZYIC3bW0bXkKsZ4RAcrCcTTFNw1HJUcpFNTQYAcNjHcRA6Hn0ri6nEUbpXBo1omALdsrgXf6PVm6RQhqht/RJYYREDdyAp/81aux3UEgG5aEVzZshOXxGXCyyBpfiCdARdlJdFrkTh+M/0j2f3VBa3d9Ijm3Fs6056Hr8mc0qF8fWjbfIm6/I6hQVx0EtM1G9Y3R+DzMBJxnxrEXJt8nHjXN7KzqY2icUkQU/hGnpacHyPv476T34XfyaMpjsj2jGgct3GGLViBdF5qNE11/kr2cEiaiF5nSFophDyowSq6LLi51wdJkC3ymWmbKvzSHHBUxpBE3qlA+4BV9Ua+E54rPkuo7CvByUg2tOZgFys6EvOmpwzByA00rAvFL6j7yRryRFu9hgTdxIr02bT3O8cygoZ5H6MHFTRzvfgHU73lFHE208aznLWJHKqE40hSeMC/J3CUM/DjPh/bpeozMFi8w3n4Yfp1/T4bcZ8OR2cvgffNjMsXIFCeKTqbLxYLwnnEYftQxIA/iX1Ezqko8Wgqx3FcZ9x2IpAMqC0l5VQCO83CkJsc2QaNsLGz3bYGKj3LYWbwao9qqabirAT5vsySd5rtAL/syK3VbEfO666jc4XPYcjoEVQ/FIGVEwDPZCQ3TKcQu7CfjGlRohl4TPtLSQechXVw8YzOYeVNy4Xw7/Rl+i+SY3OIylCPxzKTN+LBWCF9p2cCCnk52j0Mm56t8nc1xmY0hG/rIWoPXtO+aPdgeFMbUpa/JzsrpsOmvr03c/Iq+vGtLa6qCybhHNrBeqwo3PjiCIBwCgf4caJ1OE6y5v5ZMvfWWJnVk4o1XMvDmn+OYo9MBZ9eUk4sWSvBO/yKnpGcNNVsDYGa2Je7WOIgP4nZxIZVvyDzReqK66yh5q5DPvrn0mZQKy9PBwSPcxjEboWBaITby3TCx5iS1/HEVBrdy2F6xC3qkXtJ7Rqtp9Ohmcj1vlAbr5wtOYhIG67+gKX98sERvA+ytWQ/P7lyg2xcM0h63YviWr4cKTzhurqUaOSKvimG/rSGY0UVV1zf0qEUXrfRm2cgLxmj8YSZAkQbZb7ISTyktpgViDjA8/wNaj1jA3dPzUEoQDFfvKGBr9QS0blzItRh5gavEUYxX7yXe2lvpHG4tfHYdh7kHFsPPHaJ44EoHwbRYcNifRCpyVOBuSxGE7pgCKhe3AxnnDKUS5/k7r8ai6yFbHPGvgzuT2nC5aga8d51GnKOssXdONzt0cTkGPO1kKv0ekzm5rmBVEghX9iKJ+nCdPNfOIzEHesnRFcdwdeIfuuxGMllb60bsnQX055QzVHJKJr7V9ocsKUNIepAAdua1+K4wBKfunYqHrDsg53kkNhUn4cr6JpxhY8oqvV0AZ5zraFVwE2v3eSaun8gncTVIzqmbYz3VJHUG1nh//hFIcS4golhCPaK+cDGRVfCxQQUXe/FJZZQoF/doPienbkwKfzqx/no7yaSVvrDtci1nkdFAOrQy6Q39E0RJfQJ5evMsvdS2Cd5sTKZL9yrCij2PiZhLAAq2m9HSLHMsmC2KDovLsWP3V0IWz6QmG2aQbMEKMm/qbDJ4YhWo/aij6iYiWKC3BY5MdKW937/TzqEvZHKIGez0Eibdz5QxbHMZoGq34NPgJ0HXuaP44I0QnNl+CladasKM7Nl4vEMIrGcr49OMrdjpcQyP7HxMdzhlYElfM+58LQ9RWsdJltx4QYRPEdqvSwfVVzF0c9Fq5m1UCn7qDYCD3rXY7dBIpa6mUL22TJSXaINxMfGstb0m5ok+IU+lWHg/WEKf3loOrV/D4XR8PLk5IQOaZNOxT6GdWEmIE8vtidiu0szdbFEAEhVDonvMsbTyAeb1NrLJs5Lgk6E6J9i3idu0SpN7bTbCGN1MQ81vKrDtwwD3rbtbUFuhg2uM3ei6mmvcmJsV3O70U5ydVwJ8eGZoel8SSXbsK67TpJHbyZ1mnY64o6v4W/Ll8336/sYltsq6Co1kGrnRagf4aZzBttc24lk3R7LU5iJ8Kp1HepDA2nNb+Ip1lVigMY+qn9EWbHjJp0s2eaF1tTluup8B3et6OFtVLdry6ya76nYvjKTe4acGPqC+LxaRkJDpqHdhWCChso9N3SKKS2OlOb0wNwje2g59cYZ4u0VeEDj/OiYMJrBtn47BqeSLxGRZBwQqeJLZ4sfJqGgsdfRogNykK4z+3Zs0fNpxVlXoEtqOmEHi9Sa2P6CEs5rkjZ11Npgu9kQwTUGTu2Lwk/QnGcMi9Tds5vUY+l52I653CCahzEkupoUPVSejqcckXbr97EJ6v+wsoP50mjJmP1RxmWxESQIcPdnERscH0qBnBdgeZ47RPXbsIo2puGy1Excx4zA8buZD0/AiEBFoQSpbxyZblRDmoDKN7ZvM/noZz+nvHMtsiMkmEs3SnHMDDynPik6umcXxPXTRR243FzhuGewREqHSlxfjCmVTdpXJI44Zb845m+iCrvM3tlNMBt8t0IN5rv3kcXYWPP92FKxDr0LH1+tQIP6KXbzyCXupNhyt7At5efNU2TkRLBQ2++Af1xSutGcWqS+JAsfdnjjdeQlbtqpe4OJ/Bv2FvanohwXsrblCuPaEM9fgSVnzjQ/YCtvr7D7nBm7VtKXE/UMhd/HDY/JOVovj5hwGAW7HuFAX3PGhjPaGppAZOfHs3o09OHgrAwOC/9B5AcshbKCEDt6rpG/nj2MGNaSINk2Hc9pqRKTjMVQXeHNua4Sxw5+jfnFpfL+kK9Q2Uo3o2YmBk/tvGm3vBZO9b7PHJSLBe/xlwdvGauLtuRha99ni/BRCM0tSoeeHgFY/kcdVTZ24oLAatpRIEZGUQ5g60QBD3yI3NWcaa1O0HvPvm+LeC/J8MbMmrqDsIRVeJIW7lx5jtaaGQfVvb3iy9wwz98BhTlnMCo5VeqJZjzo4bZUh40Uj8adnHlfkuQOdyxRJpq8zTJVoZ+UMpoH+GEPcqqiMBpnjm3ca2eI5QQ6b/b2OeK9cSiT2b8eSZc5w+8NHbsMNK7hodI6Vi9MijZpyeO9SK+8ho8OskHDgivYuhoxp0bjl1W3ulCYDCTbxNGUwAGrJZJrXoYT1HyTYwbXiVGuGIjfKnMS7RnswZfNM1HNJhjknS6HnSh01ztcCzXXzOH8nHVxWvoIcej7MuceG0wWTrjG361qZUz7rqUV+F5rvrOGyxIpY0f6pYCbqiOVnMwVaQrY0KqMcRu8TonEwlJMK8ID0JnF4/M9B4PV0w82X8mSaoguRlNcE80AQiPmpoLpYDhv4RYm4/HlJwioMISxRDY47eGBr40RiYBSC542OwmtPFXpXX5s255cjfn2Hm/vHsq3Nk4iR6R7u7vqp7HdNKxI1YzxRyZtLz7vnY47Le0GPpj6tje1hZWcZ0eYFJdD9cA7mLBYlxc+aaNGlQ+xiqR0oXdiOztuzyDNXESJF+on85Bs0bp87Zm17ROyqe+m8UVv2FeNKXoyeQZnjFk1/XjzlZxg0opekEH5/O505sC2QNX4aQ81cNnNR7+PZ3A0CVF2bxOWaqTIO0T4krMsN7Vty4OJSZ4yeewZVg5VAosWU5HueAeneJtbZiaOqc+uZqQdXQsihA3hy2nSqeEOR+yHlBjbNtzmzWxPR5YsE++d0LLqXXwBJY2nyy9wJYrYGcF92byQG8wBTHdZzGy0rcSRZg7HctgLuOJjQEzek6D8SlrCu7jgXGRIHLmsnk18lC+nUa7NZ1Xox+J4fwSlHjaUf79ph7tH7xMJwESt+UolN0S3BgZ8ENXLf8hxGG1iN8dqs7bA5ev2Sgn0hKQK1KYOC6LmmMNGhoGkg4TJXffEYr9/+HjRfqsPBT585qfRykP5nKn20uYz7c6yKM8hwZBZIn8J5W1uxXOU3qdmSSca3W7P3v2vCtQsaglM27yjrf4J7N38O/JpiizMdOfgmrs2vPRQKDRdTmrbFnCYeF9qoA01HlBkDR8+Wwp7PhnRngzO7LcSBy8yKoj46XwRrNZWp+N6zrO/yTLRrGRV8farNubnsw6qem3xLpgDHvLJDh3tX0OG/wBbnzzU6/u+wxZL/CVuo/CdsofJ/D1s4B+7+L6jFXJP/X9RCQ3JArB4+ba6HGS1qyGd+cWOqAmhQ/QTEzlp6bmwdb46CNz5+JElXH0pkhPiu3GD8a7ZzWJGVT1QgimQCW7EkgfeoTZmt/gSCVLfnjG+YKO/BpvnMdqle/oz843z7kTzmer4DbNCYA7PylVFUag26FUfgiWAT7KgPph7ePlDs7E/UbHeAXE417LDjWM/8v6Hnt5fcudFc6lbbTyL/XKBJrscwVPgTfTsgDefCv9PEY/NQSamX9bKURaUmc0xfcw3CTPNoYq0HPlmZD1erf/9VlWryLayMvZJth3cTlWH7j3ou8VQyG9LgwhXuOcek9E5iD/UHc9dqo1jHbeKc54MuZkv4bcFdXwNm17RzzAlvytOYaMXMPjWRUzOdykif4cH5ovNk3t80/fzIAihlf5DhJ7LwaEwb+bkgDdb0niU+JZ/IwIEQOBYnCaJJSbTYNBkyDi2BbW+7WJWLheSTmR+4roiCowp25MJtFudM7iEmu50hSWsteai1idtYuwz8dF5y7ovNsN9xJ+4ZNSdVfbqEqr9j1xedgG1z7MAv1pOeiSmjqTtvcN9500GtbgVWXajAtetn4t7wDUTrK0e2i0aSodpvAvmDlqgQEkyDasPhjV4sFIS8I9qfLSB/sRqs/DwfTn4yxUd3HHBD8VI4s/8gq+e7Dws265D0NC0471KF13+9I7ptMzDTRgbFN0QS3dIG4uxnSxWLwiFpy170+LGcPkpg0fZhGMkb1Cd7z8uAZNoIdetcDCPdE2D3wzj2xfsDKBSUgF2t2axTOIO5nyRpQsV0yDx7DlZvIfTXgDQqhKnRNxV+0Gz9mm2THWQMChLxtLMfqZy1Gnp3/eb6JutAht1a8mB4gD79k8MrmGSCIXNt4VDwca7gZh2kjJOHMS636YwNUmjfEANFnpqQs0oYdCQauWcO0VTE9gJ94bAHhMPjwFX7KvHfOo5dM+wpyOmXoxpfOTrf/g5VzJoEQzezQPvQb6J8tppdEsVjDk+1Ap5DKj5UG0+NQk7S1MJ50LEljZQ6zMe5d+vZ66/i8cXn7XDhwEaaeuogZBXmEd0J6URh8yky50YMKL3KwBGrJOr3WRhfeDyj7bqTIbLpOxczwRymeBXRmy8k4GfpKtAmzphz3RVUXz4n/rumo19WFq5SSCWDj2dB65wbNC1SiD1OjuGlob147esXuvuqBAze4GEuq47lry6QxUsT2flv45HIDXEBNStRIm8LG3fHEDOlEtDPx4y6xdjB9ZibpFyQATm4j8ndIMof7FsBdy1OgIZnIXPthQx1jjHmUkROMB/SyuHw4SoI9pdkZR3O0Ym67fy94/L51xa95zYxr5n3Xs0g8yUc0v002aQgObzu0s/v11vPmCuvBMdDj6j/gzS47s38TbVVScgkLyzzy4d6ZUpj+iRpSbw4uySvlpTp1WDq22TwmyyNB1LO0h/Zx9nIK8OCOKkTrGqVCESv4qPWGmNwDpyABeISONfekOwYG8/zCK0FH+EIEtlTid2R6dRImUfcZwhhrG8QuCxfDd7D9uTegkLGMvUo25o7hgTuVaNTv26lGzZs5rz0tbmbHWpM4Qc1XpTjRUHC8ARO//U73rcBXfbh8m98/6r6hUWeonxRugJkpMzwhJAY/l62GifHJkJYqCEc+d5MNX7Yguk2BVL0fRdKxhdjTlsTqxO1Bk5GtLKfRhdQj5+aUKIYTWamHoZG3wVEPU8XDUkyrabmaOD5htHw/HsmDTtx/jNr+LDAE/T2m4HQK09aI3uJKt3Mh2+thsTEI5+u1p2Hpt3CuHinBu7JcIXAS3J45NUqeu13Jl6d9464Zd8kt+44YtUBEZS9KkoHDC2w4/wfZprYLhQ7SGDk/iFySqQLdj9aC5IJ0uDCpGFA8gQc+viCZL5zp0PvO6lQ6GnS9PEz26/awnFuIgvTiq5yQ4YNTIPETd7o5UzB60k23PSOa0zO8EfeijpOIKexmzdGUMpczukTnE9WZopEHvBEs+7CKt3zuHu3HP0W/Y3ofq2jt7ImkDT3z1T4ayGhLqM0eHefwGbZlb92OspcTmS5fXOSBUsbRBmH99NZS/csgbSJiMDKcAFTeiKAlyD8irlnd0HwUDqJ/zFtLf/F2gZmrPBS0FyxG/H2HHByTKae7GcywysbL+qWMS53z9LG4cW456kkJAqZQWecGs71XYjSkxbSf3yyMGSJCoh0SeGpeifouz9CXrq/IIZneLieShHfhSvQJWYLvN0dT0ItzEH9WjqxyYvApQfH4uXUJFp9WwqPfvSDTzQYNCNMiLTxQuKmrwvtSmagy6jiqPIpPMR00z2K95rMR7K42ENddIxRIM68ZIiz9sWzDeuLYdRHBSrXNjOdCgOCtCW++KCqmCaV6MPSsWPReUsuyurqQ8LBZ3Qo7Tc9rTFCI1tOcs0/kwUb+jiOqXRj7ii3szWlOTxD2TMsflVjWtQG+ZNX5wtOzAzlPVVxYy9LGfBsHz9kHCImC4zap7Aba21BnBVFn6QT+HOvNNbLMmScpg9UG/rA++Q4Sr58pDOmVrFbz+uw1QM8qlF0hkso49gBkQi2m/+TZ2lis3BdRibfRXM2F3FfjB0tX84su6HFVQ3k8k02R/NHj33nVe9eBuWh0bCuexbpOqmOV75koGhbMTF+zeKRABMq16aFE7rC2PTMO1T5+0b0X0KplIYY7FQIoOb+jfDyr198+WsV8/SKFQgJPtE9s2QgblkKud96g9QtHyGbtyzEd+gIE5O24I/XS8m5YRc49qAG5d9VUrvHC7EyOJf88u7lhrdvx0vzJ2JAO4DRyGs68bshah9ewknOSscpe9eTzUvaaaCmK7TtFIbe/VLwKPINjQ9IZbW+fKOnYquwe+UJsPBPwLi3ZeSyXyTK3gkCv6s36C2Ni9QhtoGeOjURk1bELOQmPGXde43YjxuOcAsHSpmJ3RncaftR9pPMWHbNASHBtBTC+Dj+ZIrXFggea89seq6bzqspH8PO3X2YG7hqA/pUwC7JHiGVRUnoV+NG74pTdob1VTrh136UKYmH1meFpGe2Dz1kUyVw37EKNxzWJwv6cyC0MR7FntbR520V5DbWCBxv8rn6gxHU1DQcjFymEOGWYxgb4Ql3C53g0enn1OltAl6YcpXe/Z5F2qzXwEe1WJgvZAOjOkWC0wtaOXUfBrVkM7Hw1CIcljqCl6W2sJHTxxKdpHxBTKAvyZkSDc9mJsHW9evI1YIocPhTysnHZ9IXyoUcLNoPniekoVn5AIYOjJL6aYuhkbcVD16cDV1xRmiSUs/suYasi94Bys/eQtRlgpjj5ebk5tESvvitBO7F7a88vhPlSgOVyBSpVMESkR5ulnCEwK8oSrCzo4Y9uyZJ0FVuDTvHyVA/uUR4ye+ni5rUacnWETqusZxNZo5gXHEqsOHlXIFTLus9QY0TyBzhMqd7cZbgTfRWCLisQ07nsyav53wcJjHiD/sExnlPBQE7prGfNbeysb+qBYf1ZXn2fcvhvtl7eq/EHH4PuWJfjynUvdHFCGFJMv9CKv4W7yGW0gxeOZoLMtn36ECUNn5eAnhhzQJwnC0l0Jk3SNSYTAx95AcT436xZd9ssHi1PdlfLQOv9dMEC9JD6abjlmD74A1ruOYK/QNJMN71IMH489zmfSLoaLQSb647CnfgElHL20Q5g3mczBWAux3Pqf/eSDxolwbnnj4kOfxKuo2Ycp9G77MtqRHMKoe98OabEZVTC8W5DTsBt5sDf+QhO4OE0GvTj+JNrx30JDnC9arNREl/UYj0y4WpzY3kww55ojm5jPseMwaoigs8MZyHRUwpHk66R1Mz7enAp242PrmclS5tI/udLNF5SBOPJJjAu0UpYD/015ckWXCOhtvwvf0y8Nx4nCOqp9nZH6UgaaQK0wzlYX5nGdetNIV4eZ/nHoh8Z0X9T4PypEwUqusm6vKHuA4xXfKyWI9GHDZiD0zTxqDUubhktALMspxwo8U+cvX3Mlj8cxraB4mijbQflLyIxv5BefDlt9Jl7mvxetBdMro0CfXZTOh6fpJVNFgNo9afuaLVI1RVfh81DtbF7u85cPfmLMoslMfp1+KpSJgr7ORKWOvwOOI9Zg7mbqqCkCeBxC5oOeh6uUPC/QGy6EQfKfzeQ1zzkuDc/Ikg1nGGbQ6cSY6FnWa7buRxEq9ciaauKZE1O8DejX7DBM6OYuUPhbL56osZ3xkOnLXLYX5fqxx7wbKUmV1rycXXroDrXnl0UGMWyHaGwC93CWyhLTQ8dSK343U2tHabg+uXWegj74iHn+jRWb2WmFwqDlNtjoFe01honq6OzyvC0eT+r7/35TzSbjgf+RFS+HqHKYxJt4LgDbuIk0UhjuvR47w/tNGPBXbwWvkhfbJpFSrdNkGLs6as0arjGPm6lt31JYN1CzrF9czVoMcLJpvmHzjPSd4vYdZsXMtzPrmAe7hHjNf+20Ow56sPR+rFuO0iK7nXYw8wuUNT2bQVS+DVphlwdXcdtanwBYnnCXDO5w49smQ87o2ZjK3NfGJonA0VxVtx6UWO+r6ZA4Y+QigqXkgnLr9M/dxnwe8tAVgY00gb5inCnxXqpGPBcjBo/8T9UMggM9O3oNuOJhhe84Jur6umFeq1uHS2GRyI/ETCOidS4d1nyDmJTGha7UZT3C9yGb+N2SWvutmMgQrWKKmV4Ws2cur8pazNFh024MVhQdLHw/yZmT5sjOh7ZovpNKa37QJf0feuoPXaCkixOY6vZSPIx0yWzP/UTCvKJFFH3Ag71+ewt9kirHYphim1dtSgaCO0eo9DJ6Gf3IQ+I1wQOwFNFfXA7rQBBCo4kAF1K9jz+BZ5nfOGE1+nD4ZBudS3KRrehFpAn6gzjJ0CwAy1k3k71CC7ORWmSeiSP6tl4JnOEL1kp0Dz26Oh6YolXLBTwNk1F0myhRNGuejiNttP9MmUCPpmhhvM3CtGa3KXYNNcT65x3xwUiY3HNVf7OGc9c9iT7IWdFTtIo8I8nHooB9bcQCriYYieP3Koj/V+ulRyKo7jTYOgmfOgvG2EWl5SgBDVcdRYNA9+FErhg8Zs2ltpjKrD48HH4gkpsOon59U0ydcbk0FzXgpKT3MAtUfVbMzBfEyKUiQi7Z28Mz66VNH+KkyuyqMd9y7wQvJCeS262lwLE0c6iwcxrl8YTme/EdD6i/wxKx4t7Dk1RlAy8yz7uqqOvrn/DOY/1kAfN29W8rgT72bYcf6EVntwqG4iAbF5pDf6KEyZYAZz360kWT3T8X3xNBhaFwtMWBIMTxoD6yYEUFPjGaj9haDa8wVwDBxh1qkMGBvLgNDGYlrtmEHrX6aRITcvtAh6Rs1LJf/+9zVy/N4FclmtSdB2KBneYTboaURjX6ktNp4xwma98SRUJQ02O8yir6oItyIklQ0SK+a0jmhQEeM+9p5kFS/Zy427550oOETSTTFsf1Psw9N8yWhRTnbJABNwNpYfs+u0QDF0Jeh3TsWuxEh8JIckNuQKtfonEfyDHKAtTgfrp3lS2LdAkCriDgFn6rBhKJ2Wd9hDZuZPctDzKO2eL0WKZV7zDqxPRe7KYUjaX0HbxabgngFtFM9Uh/b0bI7aWIDfXlloHNyI4yRlofmWDMTyN+ONjI/kselmlJLK5At1HSdVxdmYoRYClrNnEh/FXTAsI8tK6pbTpT9XCH61KOGe59Xw0HqIhh/Ww4NO76nQXHlO2kILICiRmuXawg3nOLghdIPO3V9Lqzpn4r6DDljwfJiWnIgkjNpOtPQlGLBfA6yab1PnbRbQDT/ZLicvGFO5B74HRMDaVgPInFzGavD2w/r8HfzYnjw6s0+CnDV3x7f5F8h03lJ4pxCEby6aYJftWjp3EiV37PPBPlmWPSQVRwp+26GC43RIjpDA7bn6f211LuxLlaWXk4qg3+sh6Yi+RmdoheDNu5QIf2mhXzeZ4YFFlVxj73oMGNTBSZrdROm+FdQYptIp8w3YS1VH4dAxFdT9ncL30DXD+SU32A29qaB+JQSv5Sdx/cqxpCSI4/SUj6D29H9Yc804yHRLxnv7Ozjjoa/E5JYiq+sxE/xsNenths34qfsmGzLZDLQirWFH20320dlokE+Xx1XbRCk5t4B4394NA12J2FiQSd6X17IXPeLoko4kMuqkA4dnXsGQp4bw87o0Hck15xq6rEiKIkfPv4kiE0r72NBbTXhhWiikqVfRVV+c2aEzHmAddZnoPB2LSVMjMf9rAPo/fkHPTO2hPfsnQU59Aukw1CV3iBKJXpUnyLRuZSupJau6Kph7XGLDsqZinLi7EXdV8ydvFhPDOIukC6ajQDBkOlEAjQ6CpWfGcwk4HntHjdiite3oNWqI0ZcOQKZ7Bqw7PY/+Dr9Fmo9loMy6dnbbU2RWdg6wvTqPOZ/Lp7nvaS5UdexK1ufJHDbpZzTbxHFs8NnLbNu1fE68yJy+v6VI5dq7TUVPnOXNXeII41TzQKvCiTiXubNNtkpkW9pHcvz0Qsw16mCyhstxQc1ZtB9yZZvaQ7GOGaLL+9Zw96+o4wez9fT+6GF6L0AbS4KKuJ2Pjf/GHT9Jpe43gdPYFRD/gOP0v5WA+OQi8HysimOkL8D6Bw/JP4WDtHL2ccyOEQZrNw12844Y7nm0BsNr2cOf9E2FpuSK8T8cu8LoDm7i0n+KcMc67wjGnf8qaNYDvuWJtYLkyjaBVPQN5vJrRcZ5awNjP07AKwlZDm+HLEEpIggD9Qu4bUbidNWHKpQ8Og3CmEnk0a5xOFzUSK16wknEBwH8DknkLx6dTh9WJ4LKQmHiL9fABtUvBKuMNPrQZxNOkZPHuAQ+q6VzArfAQbo0dQmJC6yBv4YBaUG7aO7WYxA4qwjGZ52glsNiOCq3hr2X/pgcuTGPRKpM5c60stwzEzM284IS3frCV5Dzq4aTLulh/CrVBOff7RZo9PJ46s25fNWKSMZK21Ow0/E8r2X9Co4f1E9W7VfBX8JetJrZRB423KL739whcQ5x1Dxanc7WF8U0fEfG3PhI3vJOkpA6cYhO7OKmXFUAsz/L8NL1OBjQekJBuo+tnOCGybYP2VdJhWTfx9XkxPBGCAkkJDTnObl0VZ9KbswlBdKF7PuNSaTzjzzBPRtgMM4GTI7UU7vxUwRrf0+htmeyac6uNbDYIhq+J46yAf4cE93YKgjvv8gN+e6kF/VLaGi7KrgvjQXOvoqEZGaxYm0a/G80Bc7uTYGP6A6L2gZI0Pk5JPOcFfB/JFNR4Tp2tp49zsmK5F5Oncn2HKrkZTt2cm/YW4LNNrKUpLtzTX2xTO2TfsG+l2O4o3PVudaFyU0J8u8Exc9yTFVtknmbIs4zwTGekBaWgQ0ND+n7Of1kk507BuxeD8n9sjisU03lbf3QN2GQTPiiCcEMRwqv/40P7urTO1bGsFV8O1K12aA1/gq95PWJ9fD8mxOpr+TOi8QQU54p+85KB2Z7GbNSqidBQlGPGHgkgeo4YZSduYmIjlkGfc3FQKVz6NrpR0mW41TOdXwAZ22oQUb4acTL7gHXtjKTHTlVzg+9Zs1vCt3Eiir3CYbyj7EizjFsfLyrQPZ6CdsUJ8HtzY0QbCqwh/ziRIje3UN7/8ZrOXMIZHs749ELY1BrajwNmLQP22VM0fuQHgoWPKVCT2whWUmRpuv4wHSbaMxOisGUsdPh3vVOzvHqQWB71Zq21JSRweMK5Ga0L+Tp5pHXg5bQqXePbtwcxaJBEnYWesD8aR7cgUpLzJD+wD2N2A3HZp7AS9N7WM+hbuLp8Y1zOBkDrV4iZLzVDJTYdQoEw6q0cNZsXOMWxDa2zILGLQakf6UFHh6s5ySbFcihLDt4uTsI5BhXMnqeBxu/F8OuvovEWXYOrmjPJAeyquiibAX4FD8NRjzNcHLEKDV3moCHL6nQsWnluOHEFJxUlkI2X1yEm+/IY8KtYdIe/5r+UrIgt7TUcZNJGjosFRYxEP2v/ab/tpT+O8cK/S8KF/73+d+7T/9PkX9rPf+Wc/6d/1Wk8e9HlvyHyFixf1kif4fP39VP79/dfwPcRU3i"

NODES, CH, DIN, DOUT, NC_ = 50000, 64, 16, 25, 8
NPN = NODES // NC_

def _factors():
    d = np.load(io.BytesIO(zlib.decompress(base64.b64decode(_WBLOB))))
    return d["V"], d["C"]   # (16,64) f32, (25,64) f32

def _np_compute(t1, t2, V, C):
    x = t1.reshape(-1, DIN); y = t2.reshape(-1, DIN)
    a = ((x + y) * 0.5) @ V
    b = ((x - y) * 0.5) @ V
    return ((a * a - b * b) @ C.T).reshape(NODES, CH, DOUT)

def kernel(tensor_1, tensor_2):
    t1 = np.asarray(tensor_1, dtype=np.float32)
    t2 = np.asarray(tensor_2, dtype=np.float32)
    V, C = _factors()
    try:
        import jax, jax.numpy as jnp
        devs = jax.devices()[:NC_]
        if len(devs) < NC_:
            raise RuntimeError("need 8 cores")
        Vj = jnp.asarray(V); Cj = jnp.asarray(C.T)

        def per_core(x, y):
            xf = x.reshape(-1, DIN); yf = y.reshape(-1, DIN)
            a = ((xf + yf) * 0.5) @ Vj
            b = ((xf - yf) * 0.5) @ Vj
            o = (a * a - b * b) @ Cj
            return o.reshape(NPN, CH, DOUT)

        pm = jax.pmap(per_core, devices=devs)
        out = pm(t1.reshape(NC_, NPN, CH, DIN), t2.reshape(NC_, NPN, CH, DIN))
        return np.asarray(out, dtype=np.float32).reshape(NODES, CH, DOUT)
    except Exception:
        return _np_compute(t1, t2, V, C).astype(np.float32)

if __name__ == "__main__":
    rng = np.random.default_rng(0)
    a = rng.standard_normal((NODES, CH, DIN), dtype=np.float32)
    b = rng.standard_normal((NODES, CH, DIN), dtype=np.float32)
    o = kernel(a, b)
    print(o.shape, o.dtype)



# revision 2
# speedup vs baseline: 1.3953x; 1.3953x over previous
"""ContractProduct3j Trainium2 kernel.

out[n,c,s] = sum_ij W[i,j,s] t1[n,c,i] t2[n,c,j], W = fused even-parity
Wigner-3j tensor (16,16,25). W is (i,j)-symmetric and admits a rank-64
symmetric (Waring) decomposition W[i,j,s] = sum_r C[s,r] V[i,r] V[j,r]
(rel. Frobenius err 2.75e-3), giving out = (t1@V * t2@V) @ C.T.

Bass kernel (8 NeuronCores, nodes sharded, 2 token streams per core):
  stage1 (PE, K=32 packed 32x32 tiles): p = x@V, q = y@V -> PSUM f32
  qcopy  (ScalarE): q -> SBUF (tensor_tensor reads at most one PSUM operand)
  mul    (VectorE): pq = p*q -> SBUF bf16
  stage2 (PE, K=64): out = pq @ C.T at 4 packed col positions -> PSUM
  outcopy(ScalarE/VectorE): -> SBUF bf16 -> DMA (spread over 3 DGE queues)
Host side: bf16 cast + feature-major transpose in, block-departition out.
Falls back to jax.pmap (same decomposition) if the bass path fails.
"""

import base64, io, zlib
import numpy as np

_WBLOB = "eNqlenk4V1/Ur6HIkDFKFBKKREXle85eSuYGQlQqZSoNhAY0GCJD5ojMc6YiKvl+z16KSpIGVEr9GjTTpEHRcPvd973De/+796797LOfvZ6zzvOcZ6+19lqftRyWio4xEPoP0hT65rW25M9/0lihCUIus/12hggLyQmFy/3HK/9jPbpi1XKHNcJCe4T26Xp5B3kG6vI0dFkfY91ZGro+/oG7Ajf5ufsHenn/y7fatD3I+y8/aMumnd5/9zPmzJ+lMd945iyNAxr/jyT5bmItDi8PwXXj20BlnCfkckHo89gXTfg+2L7DFxPTdsDIvQO4ujcIJlwLRr7CKfioEwwq9v4on3obizzvQahiENL5vujYuhWdP9lgV/EuHBDZh8t2lUNZnz9om+yDcMMQ7Kqwx6/ye8EtYCt8F/LF5o5ALPfYA/uVK/FbzF785lyIldd8cezqcxgxuhdUirah7WVf9LkcjLM89uKGRVdJVXEoBO7ege5yAbBSMRCznu5ARd4OrPTzwSMiahjxKRgDlpXh/Ru7wOpbDWiPVcZ/lBRhe1AKGg2E4dxb1fBiYjC6yvjh21mBWGxah+PnrMRtU/Oot9gjkjA0htwbZ4p5Z+YitbjILi89iT9X7YNS7itZfU8UjqeGE1TxxgOxMTgY6IY+MXxUCUqDiZKnaYFbLh1VGAsRCXn42jkeNsQkoL/PQ1YoXBw/VkphyTxV8uytDI5EjCMzwr5x3rl+bEbQYkxct4euNxaQ13QBZ2pkh2pPnpD+M+dwktgBdDOLA2WvG1zZtsf02KRx3Aw3gqyzAYqfLgUSnwINPSfJYlEhuiaqi75aFc4aXWnC35Ga0Cq+FXTVbGH9cidYzSyh1nvT8H5dGRicnEXqYivQmy8E8pEpaLpqIkrFzUPl6Rsh+U04Ks71RNywBro6K1nt65dA4KRLrsWFg8zWBgw5dgpEp1TAftSEAX8v+LVYEqqHlcn4oy5wzT8Xty2biVl+FXh601LytnMPaXxRRmVTlHD52RPAWOxD6ZMpsOuvntTJDBCBxjGauyEBi5SnAG51hAnK9vDs6AOiirepdk8etbI/TiMSPOGzWCJZ08XhonBKWvOTcUJkLlx4HIOlIj/Ii9rVeLFhJfssuQAWmo3D2LxbIJl7gFwrbIGaphT4vnYfST5XToXvD1Mr4w3wpOU9NTNIxbHav+m71fmsTHA4ebtMnBbbRKC6XB62bXPHWz7JaKWzDvc8yMQJURowOr2LuI+XAN8PZYIC3bW0bXkKsZ4RAcrCcTTFNw1HJUcpFNTQYAcNjHcRA6Hn0ri6nEUbpXBo1omALdsrgXf6PVm6RQhqht/RJYYREDdyAp/81aux3UEgG5aEVzZshOXxGXCyyBpfiCdARdlJdFrkTh+M/0j2f3VBa3d9Ijm3Fs6056Hr8mc0qF8fWjbfIm6/I6hQVx0EtM1G9Y3R+DzMBJxnxrEXJt8nHjXN7KzqY2icUkQU/hGnpacHyPv476T34XfyaMpjsj2jGgct3GGLViBdF5qNE11/kr2cEiaiF5nSFophDyowSq6LLi51wdJkC3ymWmbKvzSHHBUxpBE3qlA+4BV9Ua+E54rPkuo7CvByUg2tOZgFys6EvOmpwzByA00rAvFL6j7yRryRFu9hgTdxIr02bT3O8cygoZ5H6MHFTRzvfgHU73lFHE208aznLWJHKqE40hSeMC/J3CUM/DjPh/bpeozMFi8w3n4Yfp1/T4bcZ8OR2cvgffNjMsXIFCeKTqbLxYLwnnEYftQxIA/iX1Ezqko8Wgqx3FcZ9x2IpAMqC0l5VQCO83CkJsc2QaNsLGz3bYGKj3LYWbwao9qqabirAT5vsySd5rtAL/syK3VbEfO666jc4XPYcjoEVQ/FIGVEwDPZCQ3TKcQu7CfjGlRohl4TPtLSQechXVw8YzOYeVNy4Xw7/Rl+i+SY3OIylCPxzKTN+LBWCF9p2cCCnk52j0Mm56t8nc1xmY0hG/rIWoPXtO+aPdgeFMbUpa/JzsrpsOmvr03c/Iq+vGtLa6qCybhHNrBeqwo3PjiCIBwCgf4caJ1OE6y5v5ZMvfWWJnVk4o1XMvDmn+OYo9MBZ9eUk4sWSvBO/yKnpGcNNVsDYGa2Je7WOIgP4nZxIZVvyDzReqK66yh5q5DPvrn0mZQKy9PBwSPcxjEboWBaITby3TCx5iS1/HEVBrdy2F6xC3qkXtJ7Rqtp9Ohmcj1vlAbr5wtOYhIG67+gKX98sERvA+ytWQ/P7lyg2xcM0h63YviWr4cKTzhurqUaOSKvimG/rSGY0UVV1zf0qEUXrfRm2cgLxmj8YSZAkQbZb7ISTyktpgViDjA8/wNaj1jA3dPzUEoQDFfvKGBr9QS0blzItRh5gavEUYxX7yXe2lvpHG4tfHYdh7kHFsPPHaJ44EoHwbRYcNifRCpyVOBuSxGE7pgCKhe3AxnnDKUS5/k7r8ai6yFbHPGvgzuT2nC5aga8d51GnKOssXdONzt0cTkGPO1kKv0ekzm5rmBVEghX9iKJ+nCdPNfOIzEHesnRFcdwdeIfuuxGMllb60bsnQX055QzVHJKJr7V9ocsKUNIepAAdua1+K4wBKfunYqHrDsg53kkNhUn4cr6JpxhY8oqvV0AZ5zraFVwE2v3eSaun8gncTVIzqmbYz3VJHUG1nh//hFIcS4golhCPaK+cDGRVfCxQQUXe/FJZZQoF/doPienbkwKfzqx/no7yaSVvrDtci1nkdFAOrQy6Q39E0RJfQJ5evMsvdS2Cd5sTKZL9yrCij2PiZhLAAq2m9HSLHMsmC2KDovLsWP3V0IWz6QmG2aQbMEKMm/qbDJ4YhWo/aij6iYiWKC3BY5MdKW937/TzqEvZHKIGez0Eibdz5QxbHMZoGq34NPgJ0HXuaP44I0QnNl+CladasKM7Nl4vEMIrGcr49OMrdjpcQyP7HxMdzhlYElfM+58LQ9RWsdJltx4QYRPEdqvSwfVVzF0c9Fq5m1UCn7qDYCD3rXY7dBIpa6mUL22TJSXaINxMfGstb0m5ok+IU+lWHg/WEKf3loOrV/D4XR8PLk5IQOaZNOxT6GdWEmIE8vtidiu0szdbFEAEhVDonvMsbTyAeb1NrLJs5Lgk6E6J9i3idu0SpN7bTbCGN1MQ81vKrDtwwD3rbtbUFuhg2uM3ei6mmvcmJsV3O70U5ydVwJ8eGZoel8SSXbsK67TpJHbyZ1mnY64o6v4W/Ll8336/sYltsq6Co1kGrnRagf4aZzBttc24lk3R7LU5iJ8Kp1HepDA2nNb+Ip1lVigMY+qn9EWbHjJp0s2eaF1tTluup8B3et6OFtVLdry6ya76nYvjKTe4acGPqC+LxaRkJDpqHdhWCChso9N3SKKS2OlOb0wNwje2g59cYZ4u0VeEDj/OiYMJrBtn47BqeSLxGRZBwQqeJLZ4sfJqGgsdfRogNykK4z+3Zs0fNpxVlXoEtqOmEHi9Sa2P6CEs5rkjZ11Npgu9kQwTUGTu2Lwk/QnGcMi9Tds5vUY+l52I653CCahzEkupoUPVSejqcckXbr97EJ6v+wsoP50mjJmP1RxmWxESQIcPdnERscH0qBnBdgeZ47RPXbsIo2puGy1Excx4zA8buZD0/AiEBFoQSpbxyZblRDmoDKN7ZvM/noZz+nvHMtsiMkmEs3SnHMDDynPik6umcXxPXTRR243FzhuGewREqHSlxfjCmVTdpXJI44Zb845m+iCrvM3tlNMBt8t0IN5rv3kcXYWPP92FKxDr0LH1+tQIP6KXbzyCXupNhyt7At5efNU2TkRLBQ2++Af1xSutGcWqS+JAsfdnjjdeQlbtqpe4OJ/Bv2FvanohwXsrblCuPaEM9fgSVnzjQ/YCtvr7D7nBm7VtKXE/UMhd/HDY/JOVovj5hwGAW7HuFAX3PGhjPaGppAZOfHs3o09OHgrAwOC/9B5AcshbKCEDt6rpG/nj2MGNaSINk2Hc9pqRKTjMVQXeHNua4Sxw5+jfnFpfL+kK9Q2Uo3o2YmBk/tvGm3vBZO9b7PHJSLBe/xlwdvGauLtuRha99ni/BRCM0tSoeeHgFY/kcdVTZ24oLAatpRIEZGUQ5g60QBD3yI3NWcaa1O0HvPvm+LeC/J8MbMmrqDsIRVeJIW7lx5jtaaGQfVvb3iy9wwz98BhTlnMCo5VeqJZjzo4bZUh40Uj8adnHlfkuQOdyxRJpq8zTJVoZ+UMpoH+GEPcqqiMBpnjm3ca2eI5QQ6b/b2OeK9cSiT2b8eSZc5w+8NHbsMNK7hodI6Vi9MijZpyeO9SK+8ho8OskHDgivYuhoxp0bjl1W3ulCYDCTbxNGUwAGrJZJrXoYT1HyTYwbXiVGuGIjfKnMS7RnswZfNM1HNJhjknS6HnSh01ztcCzXXzOH8nHVxWvoIcej7MuceG0wWTrjG361qZUz7rqUV+F5rvrOGyxIpY0f6pYCbqiOVnMwVaQrY0KqMcRu8TonEwlJMK8ID0JnF4/M9B4PV0w82X8mSaoguRlNcE80AQiPmpoLpYDhv4RYm4/HlJwioMISxRDY47eGBr40RiYBSC542OwmtPFXpXX5s255cjfn2Hm/vHsq3Nk4iR6R7u7vqp7HdNKxI1YzxRyZtLz7vnY47Le0GPpj6tje1hZWcZ0eYFJdD9cA7mLBYlxc+aaNGlQ+xiqR0oXdiOztuzyDNXESJF+on85Bs0bp87Zm17ROyqe+m8UVv2FeNKXoyeQZnjFk1/XjzlZxg0opekEH5/O505sC2QNX4aQ81cNnNR7+PZ3A0CVF2bxOWaqTIO0T4krMsN7Vty4OJSZ4yeewZVg5VAosWU5HueAeneJtbZiaOqc+uZqQdXQsihA3hy2nSqeEOR+yHlBjbNtzmzWxPR5YsE++d0LLqXXwBJY2nyy9wJYrYGcF92byQG8wBTHdZzGy0rcSRZg7HctgLuOJjQEzek6D8SlrCu7jgXGRIHLmsnk18lC+nUa7NZ1Xox+J4fwSlHjaUf79ph7tH7xMJwESt+UolN0S3BgZ8ENXLf8hxGG1iN8dqs7bA5ev2Sgn0hKQK1KYOC6LmmMNGhoGkg4TJXffEYr9/+HjRfqsPBT585qfRykP5nKn20uYz7c6yKM8hwZBZIn8J5W1uxXOU3qdmSSca3W7P3v2vCtQsaglM27yjrf4J7N38O/JpiizMdOfgmrs2vPRQKDRdTmrbFnCYeF9qoA01HlBkDR8+Wwp7PhnRngzO7LcSBy8yKoj46XwRrNZWp+N6zrO/yTLRrGRV8farNubnsw6qem3xLpgDHvLJDh3tX0OG/wBbnzzU6/u+wxZL/CVuo/CdsofJ/D1s4B+7+L6jFXJP/X9RCQ3JArB4+ba6HGS1qyGd+cWOqAmhQ/QTEzlp6bmwdb46CNz5+JElXH0pkhPiu3GD8a7ZzWJGVT1QgimQCW7EkgfeoTZmt/gSCVLfnjG+YKO/BpvnMdqle/oz843z7kTzmer4DbNCYA7PylVFUag26FUfgiWAT7KgPph7ePlDs7E/UbHeAXE417LDjWM/8v6Hnt5fcudFc6lbbTyL/XKBJrscwVPgTfTsgDefCv9PEY/NQSamX9bKURaUmc0xfcw3CTPNoYq0HPlmZD1erf/9VlWryLayMvZJth3cTlWH7j3ou8VQyG9LgwhXuOcek9E5iD/UHc9dqo1jHbeKc54MuZkv4bcFdXwNm17RzzAlvytOYaMXMPjWRUzOdykif4cH5ovNk3t80/fzIAihlf5DhJ7LwaEwb+bkgDdb0niU+JZ/IwIEQOBYnCaJJSbTYNBkyDi2BbW+7WJWLheSTmR+4roiCowp25MJtFudM7iEmu50hSWsteai1idtYuwz8dF5y7ovNsN9xJ+4ZNSdVfbqEqr9j1xedgG1z7MAv1pOeiSmjqTtvcN9500GtbgVWXajAtetn4t7wDUTrK0e2i0aSodpvAvmDlqgQEkyDasPhjV4sFIS8I9qfLSB/sRqs/DwfTn4yxUd3HHBD8VI4s/8gq+e7Dws265D0NC0471KF13+9I7ptMzDTRgbFN0QS3dIG4uxnSxWLwiFpy170+LGcPkpg0fZhGMkb1Cd7z8uAZNoIdetcDCPdE2D3wzj2xfsDKBSUgF2t2axTOIO5nyRpQsV0yDx7DlZvIfTXgDQqhKnRNxV+0Gz9mm2THWQMChLxtLMfqZy1Gnp3/eb6JutAht1a8mB4gD79k8MrmGSCIXNt4VDwca7gZh2kjJOHMS636YwNUmjfEANFnpqQs0oYdCQauWcO0VTE9gJ94bAHhMPjwFX7KvHfOo5dM+wpyOmXoxpfOTrf/g5VzJoEQzezQPvQb6J8tppdEsVjDk+1Ap5DKj5UG0+NQk7S1MJ50LEljZQ6zMe5d+vZ66/i8cXn7XDhwEaaeuogZBXmEd0J6URh8yky50YMKL3KwBGrJOr3WRhfeDyj7bqTIbLpOxczwRymeBXRmy8k4GfpKtAmzphz3RVUXz4n/rumo19WFq5SSCWDj2dB65wbNC1SiD1OjuGlob147esXuvuqBAze4GEuq47lry6QxUsT2flv45HIDXEBNStRIm8LG3fHEDOlEtDPx4y6xdjB9ZibpFyQATm4j8ndIMof7FsBdy1OgIZnIXPthQx1jjHmUkROMB/SyuHw4SoI9pdkZR3O0Ym67fy94/L51xa95zYxr5n3Xs0g8yUc0v002aQgObzu0s/v11vPmCuvBMdDj6j/gzS47s38TbVVScgkLyzzy4d6ZUpj+iRpSbw4uySvlpTp1WDq22TwmyyNB1LO0h/Zx9nIK8OCOKkTrGqVCESv4qPWGmNwDpyABeISONfekOwYG8/zCK0FH+EIEtlTid2R6dRImUfcZwhhrG8QuCxfDd7D9uTegkLGMvUo25o7hgTuVaNTv26lGzZs5rz0tbmbHWpM4Qc1XpTjRUHC8ARO//U73rcBXfbh8m98/6r6hUWeonxRugJkpMzwhJAY/l62GifHJkJYqCEc+d5MNX7Yguk2BVL0fRdKxhdjTlsTqxO1Bk5GtLKfRhdQj5+aUKIYTWamHoZG3wVEPU8XDUkyrabmaOD5htHw/HsmDTtx/jNr+LDAE/T2m4HQK09aI3uJKt3Mh2+thsTEI5+u1p2Hpt3CuHinBu7JcIXAS3J45NUqeu13Jl6d9464Zd8kt+44YtUBEZS9KkoHDC2w4/wfZprYLhQ7SGDk/iFySqQLdj9aC5IJ0uDCpGFA8gQc+viCZL5zp0PvO6lQ6GnS9PEz26/awnFuIgvTiq5yQ4YNTIPETd7o5UzB60k23PSOa0zO8EfeijpOIKexmzdGUMpczukTnE9WZopEHvBEs+7CKt3zuHu3HP0W/Y3ofq2jt7ImkDT3z1T4ayGhLqM0eHefwGbZlb92OspcTmS5fXOSBUsbRBmH99NZS/csgbSJiMDKcAFTeiKAlyD8irlnd0HwUDqJ/zFtLf/F2gZmrPBS0FyxG/H2HHByTKae7GcywysbL+qWMS53z9LG4cW456kkJAqZQWecGs71XYjSkxbSf3yyMGSJCoh0SeGpeifouz9CXrq/IIZneLieShHfhSvQJWYLvN0dT0ItzEH9WjqxyYvApQfH4uXUJFp9WwqPfvSDTzQYNCNMiLTxQuKmrwvtSmagy6jiqPIpPMR00z2K95rMR7K42ENddIxRIM68ZIiz9sWzDeuLYdRHBSrXNjOdCgOCtCW++KCqmCaV6MPSsWPReUsuyurqQ8LBZ3Qo7Tc9rTFCI1tOcs0/kwUb+jiOqXRj7ii3szWlOTxD2TMsflVjWtQG+ZNX5wtOzAzlPVVxYy9LGfBsHz9kHCImC4zap7Aba21BnBVFn6QT+HOvNNbLMmScpg9UG/rA++Q4Sr58pDOmVrFbz+uw1QM8qlF0hkso49gBkQi2m/+TZ2lis3BdRibfRXM2F3FfjB0tX84su6HFVQ3k8k02R/NHj33nVe9eBuWh0bCuexbpOqmOV75koGhbMTF+zeKRABMq16aFE7rC2PTMO1T5+0b0X0KplIYY7FQIoOb+jfDyr198+WsV8/SKFQgJPtE9s2QgblkKud96g9QtHyGbtyzEd+gIE5O24I/XS8m5YRc49qAG5d9VUrvHC7EyOJf88u7lhrdvx0vzJ2JAO4DRyGs68bshah9ewknOSscpe9eTzUvaaaCmK7TtFIbe/VLwKPINjQ9IZbW+fKOnYquwe+UJsPBPwLi3ZeSyXyTK3gkCv6s36C2Ni9QhtoGeOjURk1bELOQmPGXde43YjxuOcAsHSpmJ3RncaftR9pPMWHbNASHBtBTC+Dj+ZIrXFggea89seq6bzqspH8PO3X2YG7hqA/pUwC7JHiGVRUnoV+NG74pTdob1VTrh136UKYmH1meFpGe2Dz1kUyVw37EKNxzWJwv6cyC0MR7FntbR520V5DbWCBxv8rn6gxHU1DQcjFymEOGWYxgb4Ql3C53g0enn1OltAl6YcpXe/Z5F2qzXwEe1WJgvZAOjOkWC0wtaOXUfBrVkM7Hw1CIcljqCl6W2sJHTxxKdpHxBTKAvyZkSDc9mJsHW9evI1YIocPhTysnHZ9IXyoUcLNoPniekoVn5AIYOjJL6aYuhkbcVD16cDV1xRmiSUs/suYasi94Bys/eQtRlgpjj5ebk5tESvvitBO7F7a88vhPlSgOVyBSpVMESkR5ulnCEwK8oSrCzo4Y9uyZJ0FVuDTvHyVA/uUR4ye+ni5rUacnWETqusZxNZo5gXHEqsOHlXIFTLus9QY0TyBzhMqd7cZbgTfRWCLisQ07nsyav53wcJjHiD/sExnlPBQE7prGfNbeysb+qBYf1ZXn2fcvhvtl7eq/EHH4PuWJfjynUvdHFCGFJMv9CKv4W7yGW0gxeOZoLMtn36ECUNn5eAnhhzQJwnC0l0Jk3SNSYTAx95AcT436xZd9ssHi1PdlfLQOv9dMEC9JD6abjlmD74A1ruOYK/QNJMN71IMH489zmfSLoaLQSb647CnfgElHL20Q5g3mczBWAux3Pqf/eSDxolwbnnj4kOfxKuo2Ycp9G77MtqRHMKoe98OabEZVTC8W5DTsBt5sDf+QhO4OE0GvTj+JNrx30JDnC9arNREl/UYj0y4WpzY3kww55ojm5jPseMwaoigs8MZyHRUwpHk66R1Mz7enAp242PrmclS5tI/udLNF5SBOPJJjAu0UpYD/015ckWXCOhtvwvf0y8Nx4nCOqp9nZH6UgaaQK0wzlYX5nGdetNIV4eZ/nHoh8Z0X9T4PypEwUqusm6vKHuA4xXfKyWI9GHDZiD0zTxqDUubhktALMspxwo8U+cvX3Mlj8cxraB4mijbQflLyIxv5BefDlt9Jl7mvxetBdMro0CfXZTOh6fpJVNFgNo9afuaLVI1RVfh81DtbF7u85cPfmLMoslMfp1+KpSJgr7ORKWOvwOOI9Zg7mbqqCkCeBxC5oOeh6uUPC/QGy6EQfKfzeQ1zzkuDc/Ikg1nGGbQ6cSY6FnWa7buRxEq9ciaauKZE1O8DejX7DBM6OYuUPhbL56osZ3xkOnLXLYX5fqxx7wbKUmV1rycXXroDrXnl0UGMWyHaGwC93CWyhLTQ8dSK343U2tHabg+uXWegj74iHn+jRWb2WmFwqDlNtjoFe01honq6OzyvC0eT+r7/35TzSbjgf+RFS+HqHKYxJt4LgDbuIk0UhjuvR47w/tNGPBXbwWvkhfbJpFSrdNkGLs6as0arjGPm6lt31JYN1CzrF9czVoMcLJpvmHzjPSd4vYdZsXMtzPrmAe7hHjNf+20Ow56sPR+rFuO0iK7nXYw8wuUNT2bQVS+DVphlwdXcdtanwBYnnCXDO5w49smQ87o2ZjK3NfGJonA0VxVtx6UWO+r6ZA4Y+QigqXkgnLr9M/dxnwe8tAVgY00gb5inCnxXqpGPBcjBo/8T9UMggM9O3oNuOJhhe84Jur6umFeq1uHS2GRyI/ETCOidS4d1nyDmJTGha7UZT3C9yGb+N2SWvutmMgQrWKKmV4Ws2cur8pazNFh024MVhQdLHw/yZmT5sjOh7ZovpNKa37QJf0feuoPXaCkixOY6vZSPIx0yWzP/UTCvKJFFH3Ag71+ewt9kirHYphim1dtSgaCO0eo9DJ6Gf3IQ+I1wQOwFNFfXA7rQBBCo4kAF1K9jz+BZ5nfOGE1+nD4ZBudS3KRrehFpAn6gzjJ0CwAy1k3k71CC7ORWmSeiSP6tl4JnOEL1kp0Dz26Oh6YolXLBTwNk1F0myhRNGuejiNttP9MmUCPpmhhvM3CtGa3KXYNNcT65x3xwUiY3HNVf7OGc9c9iT7IWdFTtIo8I8nHooB9bcQCriYYieP3Koj/V+ulRyKo7jTYOgmfOgvG2EWl5SgBDVcdRYNA9+FErhg8Zs2ltpjKrD48HH4gkpsOon59U0ydcbk0FzXgpKT3MAtUfVbMzBfEyKUiQi7Z28Mz66VNH+KkyuyqMd9y7wQvJCeS262lwLE0c6iwcxrl8YTme/EdD6i/wxKx4t7Dk1RlAy8yz7uqqOvrn/DOY/1kAfN29W8rgT72bYcf6EVntwqG4iAbF5pDf6KEyZYAZz360kWT3T8X3xNBhaFwtMWBIMTxoD6yYEUFPjGaj9haDa8wVwDBxh1qkMGBvLgNDGYlrtmEHrX6aRITcvtAh6Rs1LJf/+9zVy/N4FclmtSdB2KBneYTboaURjX6ktNp4xwma98SRUJQ02O8yir6oItyIklQ0SK+a0jmhQEeM+9p5kFS/Zy427550oOETSTTFsf1Psw9N8yWhRTnbJABNwNpYfs+u0QDF0Jeh3TsWuxEh8JIckNuQKtfonEfyDHKAtTgfrp3lS2LdAkCriDgFn6rBhKJ2Wd9hDZuZPctDzKO2eL0WKZV7zDqxPRe7KYUjaX0HbxabgngFtFM9Uh/b0bI7aWIDfXlloHNyI4yRlofmWDMTyN+ONjI/kselmlJLK5At1HSdVxdmYoRYClrNnEh/FXTAsI8tK6pbTpT9XCH61KOGe59Xw0HqIhh/Ww4NO76nQXHlO2kILICiRmuXawg3nOLghdIPO3V9Lqzpn4r6DDljwfJiWnIgkjNpOtPQlGLBfA6yab1PnbRbQDT/ZLicvGFO5B74HRMDaVgPInFzGavD2w/r8HfzYnjw6s0+CnDV3x7f5F8h03lJ4pxCEby6aYJftWjp3EiV37PPBPlmWPSQVRwp+26GC43RIjpDA7bn6f211LuxLlaWXk4qg3+sh6Yi+RmdoheDNu5QIf2mhXzeZ4YFFlVxj73oMGNTBSZrdROm+FdQYptIp8w3YS1VH4dAxFdT9ncL30DXD+SU32A29qaB+JQSv5Sdx/cqxpCSI4/SUj6D29H9Yc804yHRLxnv7Ozjjoa/E5JYiq+sxE/xsNenths34qfsmGzLZDLQirWFH20320dlokE+Xx1XbRCk5t4B4394NA12J2FiQSd6X17IXPeLoko4kMuqkA4dnXsGQp4bw87o0Hck15xq6rEiKIkfPv4kiE0r72NBbTXhhWiikqVfRVV+c2aEzHmAddZnoPB2LSVMjMf9rAPo/fkHPTO2hPfsnQU59Aukw1CV3iBKJXpUnyLRuZSupJau6Kph7XGLDsqZinLi7EXdV8ydvFhPDOIukC6ajQDBkOlEAjQ6CpWfGcwk4HntHjdiite3oNWqI0ZcOQKZ7Bqw7PY/+Dr9Fmo9loMy6dnbbU2RWdg6wvTqPOZ/Lp7nvaS5UdexK1ufJHDbpZzTbxHFs8NnLbNu1fE68yJy+v6VI5dq7TUVPnOXNXeII41TzQKvCiTiXubNNtkpkW9pHcvz0Qsw16mCyhstxQc1ZtB9yZZvaQ7GOGaLL+9Zw96+o4wez9fT+6GF6L0AbS4KKuJ2Pjf/GHT9Jpe43gdPYFRD/gOP0v5WA+OQi8HysimOkL8D6Bw/JP4WDtHL2ccyOEQZrNw12844Y7nm0BsNr2cOf9E2FpuSK8T8cu8LoDm7i0n+KcMc67wjGnf8qaNYDvuWJtYLkyjaBVPQN5vJrRcZ5awNjP07AKwlZDm+HLEEpIggD9Qu4bUbidNWHKpQ8Og3CmEnk0a5xOFzUSK16wknEBwH8DknkLx6dTh9WJ4LKQmHiL9fABtUvBKuMNPrQZxNOkZPHuAQ+q6VzArfAQbo0dQmJC6yBv4YBaUG7aO7WYxA4qwjGZ52glsNiOCq3hr2X/pgcuTGPRKpM5c60stwzEzM284IS3frCV5Dzq4aTLulh/CrVBOff7RZo9PJ46s25fNWKSMZK21Ow0/E8r2X9Co4f1E9W7VfBX8JetJrZRB423KL739whcQ5x1Dxanc7WF8U0fEfG3PhI3vJOkpA6cYhO7OKmXFUAsz/L8NL1OBjQekJBuo+tnOCGybYP2VdJhWTfx9XkxPBGCAkkJDTnObl0VZ9KbswlBdKF7PuNSaTzjzzBPRtgMM4GTI7UU7vxUwRrf0+htmeyac6uNbDYIhq+J46yAf4cE93YKgjvv8gN+e6kF/VLaGi7KrgvjQXOvoqEZGaxYm0a/G80Bc7uTYGP6A6L2gZI0Pk5JPOcFfB/JFNR4Tp2tp49zsmK5F5Oncn2HKrkZTt2cm/YW4LNNrKUpLtzTX2xTO2TfsG+l2O4o3PVudaFyU0J8u8Exc9yTFVtknmbIs4zwTGekBaWgQ0ND+n7Of1kk507BuxeD8n9sjisU03lbf3QN2GQTPiiCcEMRwqv/40P7urTO1bGsFV8O1K12aA1/gq95PWJ9fD8mxOpr+TOi8QQU54p+85KB2Z7GbNSqidBQlGPGHgkgeo4YZSduYmIjlkGfc3FQKVz6NrpR0mW41TOdXwAZ22oQUb4acTL7gHXtjKTHTlVzg+9Zs1vCt3Eiir3CYbyj7EizjFsfLyrQPZ6CdsUJ8HtzY0QbCqwh/ziRIje3UN7/8ZrOXMIZHs749ELY1BrajwNmLQP22VM0fuQHgoWPKVCT2whWUmRpuv4wHSbaMxOisGUsdPh3vVOzvHqQWB71Zq21JSRweMK5Ga0L+Tp5pHXg5bQqXePbtwcxaJBEnYWesD8aR7cgUpLzJD+wD2N2A3HZp7AS9N7WM+hbuLp8Y1zOBkDrV4iZLzVDJTYdQoEw6q0cNZsXOMWxDa2zILGLQakf6UFHh6s5ySbFcihLDt4uTsI5BhXMnqeBxu/F8OuvovEWXYOrmjPJAeyquiibAX4FD8NRjzNcHLEKDV3moCHL6nQsWnluOHEFJxUlkI2X1yEm+/IY8KtYdIe/5r+UrIgt7TUcZNJGjosFRYxEP2v/ab/tpT+O8cK/S8KF/73+d+7T/9PkX9rPf+Wc/6d/1Wk8e9HlvyHyFixf1kif4fP39VP79/dfwPcRU3i"


NC_ = 8
NODES, CH, DIN, DOUT = 50000, 64, 16, 25
TOK = NODES * CH
TPC = TOK // NC_
NSTREAM = 2
TPS = TPC // NSTREAM


def _factors():
    d = np.load(io.BytesIO(zlib.decompress(base64.b64decode(_WBLOB))))
    return d["V"], d["C"]   # (16,64) f32, (25,64) f32


def _np_compute(t1, t2, V, C):
    x = t1.reshape(-1, DIN)
    y = t2.reshape(-1, DIN)
    out = np.empty((TOK, DOUT), dtype=np.float32)
    for i in range(0, TOK, 262144):
        sl = slice(i, min(i + 262144, TOK))
        pq = (x[sl] @ V) * (y[sl] @ V)
        out[sl] = pq @ C.T
    return out.reshape(NODES, CH, DOUT)


# ----------------------------- bass path ------------------------------------

FD = 512                    # matmul free size (one psum bank)
PTOK = 2 * FD               # tokens/stream per pair
NPAIR = -(-TPS // PTOK)     # 196
TPS_PAD = NPAIR * PTOK      # 200704
IGP = 7                     # pairs per input DMA group
OGP = 14                    # pairs per output DMA group
OC_DVE_EVERY = 2            # every 2nd outcopy goes to VectorE

_BASS_CACHE = {}


def _build_nc():
    import concourse.mybir as mybir
    from concourse import bacc
    from concourse.tile import TileContext

    dt = mybir.dt
    nc = bacc.Bacc(None, target_bir_lowering=False)
    u = nc.dram_tensor("u", [NSTREAM, 32, TPS_PAD], dt.bfloat16, kind="ExternalInput")
    w_sp = nc.dram_tensor("w_sp", [128, 64], dt.bfloat16, kind="ExternalInput")
    w_sq = nc.dram_tensor("w_sq", [128, 64], dt.bfloat16, kind="ExternalInput")
    w_ct = nc.dram_tensor("w_ct", [128, 32], dt.bfloat16, kind="ExternalInput")
    o = nc.dram_tensor("o", [4, DOUT, NPAIR * FD], dt.bfloat16, kind="ExternalOutput")

    with TileContext(nc) as tc:
        with (
            tc.tile_pool(name="wpool", bufs=1) as wpool,
            tc.tile_pool(name="upool", bufs=3) as upool,
            tc.tile_pool(name="qspool", bufs=4) as qspool,
            tc.tile_pool(name="pqpool", bufs=4) as pqpool,
            tc.tile_pool(name="obpool", bufs=3) as obpool,
            tc.tile_pool(name="ppsum", bufs=3, space="PSUM") as ppsum,
            tc.tile_pool(name="qpsum", bufs=3, space="PSUM") as qpsum,
            tc.tile_pool(name="opsum", bufs=2, space="PSUM") as opsum,
        ):
            sp_t = wpool.tile([128, 64], dt.bfloat16, tag="sp")
            sq_t = wpool.tile([128, 64], dt.bfloat16, tag="sq")
            ct_t = wpool.tile([128, 32], dt.bfloat16, tag="ct")
            nc.sync.dma_start(out=sp_t[:, :], in_=w_sp[:, :])
            nc.sync.dma_start(out=sq_t[:, :], in_=w_sq[:, :])
            nc.sync.dma_start(out=ct_t[:, :], in_=w_ct[:, :])

            u_t = None
            ob_t = None
            hist = []

            def emit_tail(item, idx):
                pq_, obt_, osl_ = item
                ob = opsum.tile([128, FD], dt.float32, tag="o")
                nc.tensor.matmul(
                    out=ob[64:96, :], lhsT=ct_t[0:64, :], rhs=pq_[0:64, 0:FD],
                    start=True, stop=True, tile_position=(0, 64))
                nc.tensor.matmul(
                    out=ob[96:128, :], lhsT=ct_t[0:64, :], rhs=pq_[0:64, FD:PTOK],
                    start=True, stop=True, tile_position=(0, 96))
                nc.tensor.matmul(
                    out=ob[32:64, :], lhsT=ct_t[64:128, :], rhs=pq_[64:128, 0:FD],
                    start=True, stop=True, tile_position=(64, 32))
                nc.tensor.matmul(
                    out=ob[0:32, :], lhsT=ct_t[64:128, :], rhs=pq_[64:128, FD:PTOK],
                    start=True, stop=True, tile_position=(64, 0))
                if idx % OC_DVE_EVERY == 0:
                    nc.vector.tensor_copy(obt_[:, osl_], ob[:, :])
                else:
                    nc.scalar.copy(out=obt_[:, osl_], in_=ob[:, :])
                if idx % OGP == OGP - 1:
                    og = idx // OGP
                    ogsl = slice(og * OGP * FD, (og + 1) * OGP * FD)
                    _oe = [nc.sync, nc.gpsimd]
                    for b in range(4):
                        _oe[b % 2].dma_start(
                            out=o[b, :, ogsl], in_=obt_[32 * b: 32 * b + DOUT, :])

            for pr in range(NPAIR):
                if pr % IGP == 0:
                    g = pr // IGP
                    gsl = slice(g * IGP * PTOK, (g + 1) * IGP * PTOK)
                    u_t = upool.tile([128, IGP * PTOK], dt.bfloat16, tag="u")
                    _ie = [nc.gpsimd, nc.sync]
                    _ie[0].dma_start(out=u_t[0:32, :], in_=u[0, :, gsl])
                    _ie[1].dma_start(out=u_t[64:96, :], in_=u[1, :, gsl])
                if pr % OGP == 0:
                    ob_t = obpool.tile([128, OGP * FD], dt.bfloat16, tag="ob")

                qs = qspool.tile([128, PTOK], dt.float32, tag="qs")
                pq = pqpool.tile([128, PTOK], dt.bfloat16, tag="pq")
                ip = pr % IGP
                for h in range(2):
                    usl = slice(ip * PTOK + h * FD, ip * PTOK + (h + 1) * FD)
                    psl = slice(h * FD, (h + 1) * FD)
                    qb = qpsum.tile([128, FD], dt.float32, tag="q")
                    nc.tensor.matmul(
                        out=qb[0:64, :], lhsT=sq_t[0:32, :], rhs=u_t[0:32, usl],
                        start=True, stop=True, tile_position=(0, 0))
                    nc.tensor.matmul(
                        out=qb[64:128, :], lhsT=sq_t[64:96, :], rhs=u_t[64:96, usl],
                        start=True, stop=True, tile_position=(64, 64))
                    nc.scalar.copy(out=qs[:, psl], in_=qb[:, :])
                    pb = ppsum.tile([128, FD], dt.float32, tag="p")
                    nc.tensor.matmul(
                        out=pb[0:64, :], lhsT=sp_t[0:32, :], rhs=u_t[0:32, usl],
                        start=True, stop=True, tile_position=(0, 0))
                    nc.tensor.matmul(
                        out=pb[64:128, :], lhsT=sp_t[64:96, :], rhs=u_t[64:96, usl],
                        start=True, stop=True, tile_position=(64, 64))
                    nc.vector.tensor_mul(pq[:, psl], pb[:, :], qs[:, psl])
                hist.append((pq, ob_t, slice((pr % OGP) * FD, (pr % OGP + 1) * FD)))
                if len(hist) > 2:
                    emit_tail(hist.pop(0), pr - 2)
            for k, item in enumerate(hist):
                emit_tail(item, NPAIR - len(hist) + k)
    nc.finalize()
    return nc


def _make_weights(V, C, BF16):
    sp = np.zeros((128, 64), dtype=np.float32)
    sq = np.zeros((128, 64), dtype=np.float32)
    for blk in range(4):
        sp[32 * blk: 32 * blk + 16, :] = V
        sq[32 * blk + 16: 32 * blk + 32, :] = V
    ct = np.zeros((128, 32), dtype=np.float32)
    ct[0:64, 0:25] = C.T
    ct[64:128, 0:25] = C.T
    return sp.astype(BF16), sq.astype(BF16), ct.astype(BF16)


def _prep_inputs(t1, t2, BF16):
    xT = np.ascontiguousarray(t1.reshape(TOK, DIN).T.astype(BF16))
    yT = np.ascontiguousarray(t2.reshape(TOK, DIN).T.astype(BF16))
    xr = xT.reshape(DIN, NC_, NSTREAM, TPS)
    yr = yT.reshape(DIN, NC_, NSTREAM, TPS)
    ins = []
    for c in range(NC_):
        uc = np.zeros((NSTREAM, 32, TPS_PAD), dtype=BF16)
        uc[:, 0:16, :TPS] = xr[:, c].transpose(1, 0, 2)
        uc[:, 16:32, :TPS] = yr[:, c].transpose(1, 0, 2)
        ins.append(uc)
    return ins


def _postprocess(outs):
    full = np.empty((TOK, DOUT), dtype=np.float32)
    for c in range(NC_):
        oc = outs[c].reshape(4, DOUT, NPAIR, FD)
        for s, (bh0, bh1) in enumerate(((2, 3), (1, 0))):
            arr = np.stack((oc[bh0], oc[bh1]), axis=0).transpose(2, 0, 3, 1)
            flat = arr.reshape(TPS_PAD, DOUT)[:TPS].astype(np.float32)
            base = c * TPC + s * TPS
            full[base: base + TPS] = flat
    return full.reshape(NODES, CH, DOUT)


def _get_bass():
    if "state" not in _BASS_CACHE:
        import sys as _sys
        if "/opt/trn_rl_repo" not in _sys.path:
            _sys.path.insert(0, "/opt/trn_rl_repo")
        import ml_dtypes
        from concourse.bass_utils import run_bass_kernel_spmd
        BF16 = ml_dtypes.bfloat16
        V, C = _factors()
        sp, sq, ct = _make_weights(V, C, BF16)
        nc = _build_nc()
        _BASS_CACHE["state"] = (nc, sp, sq, ct, BF16, run_bass_kernel_spmd)
    return _BASS_CACHE["state"]


def kernel(tensor_1, tensor_2):
    t1 = np.asarray(tensor_1, dtype=np.float32)
    t2 = np.asarray(tensor_2, dtype=np.float32)
    try:
        nc, sp, sq, ct, BF16, run_spmd = _get_bass()
        u_list = _prep_inputs(t1, t2, BF16)
        in_maps = [
            {"u": u_list[c], "w_sp": sp, "w_sq": sq, "w_ct": ct}
            for c in range(NC_)
        ]
        res = run_spmd(nc, in_maps, core_ids=list(range(NC_)))
        return _postprocess([r["o"] for r in res.results])
    except Exception:
        V, C = _factors()
        return _np_compute(t1, t2, V, C)


if __name__ == "__main__":
    rng = np.random.default_rng(0)
    a = rng.standard_normal((NODES, CH, DIN), dtype=np.float32)
    b = rng.standard_normal((NODES, CH, DIN), dtype=np.float32)
    o = kernel(a, b)
    print(o.shape, o.dtype)


# revision 3
# speedup vs baseline: 53.4927x; 38.3379x over previous
"""ContractProduct3j Trainium2 kernel.

out[n,c,s] = sum_ij W[i,j,s] t1[n,c,i] t2[n,c,j], W = fused even-parity
Wigner-3j tensor (16,16,25). W is (i,j)-symmetric and admits a rank-64
symmetric (Waring) decomposition W[i,j,s] = sum_r C[s,r] V[i,r] V[j,r]
(rel. Frobenius err 2.75e-3), giving out = (t1@V * t2@V) @ C.T.

Bass kernel (8 NeuronCores, nodes sharded, 2 token streams per core):
  stage1 (PE, K=32 packed 32x32 tiles): p = x@V, q = y@V -> PSUM f32
  qcopy  (ScalarE): q -> SBUF (tensor_tensor reads at most one PSUM operand)
  mul    (VectorE): pq = p*q -> SBUF bf16
  stage2 (PE, K=64): out = pq @ C.T at 4 packed col positions -> PSUM
  outcopy(ScalarE/VectorE): -> SBUF bf16 -> DMA (spread over 3 DGE queues)
Host side: bf16 cast + feature-major transpose in, block-departition out.
Falls back to jax.pmap (same decomposition) if the bass path fails.
"""

import base64, io, zlib
import numpy as np

_WBLOB = "eNqlenk4V1/Ur6HIkDFKFBKKREXle85eSuYGQlQqZSoNhAY0GCJD5ojMc6YiKvl+z16KSpIGVEr9GjTTpEHRcPvd973De/+796797LOfvZ6zzvOcZ6+19lqftRyWio4xEPoP0hT65rW25M9/0lihCUIus/12hggLyQmFy/3HK/9jPbpi1XKHNcJCe4T26Xp5B3kG6vI0dFkfY91ZGro+/oG7Ajf5ufsHenn/y7fatD3I+y8/aMumnd5/9zPmzJ+lMd945iyNAxr/jyT5bmItDi8PwXXj20BlnCfkckHo89gXTfg+2L7DFxPTdsDIvQO4ujcIJlwLRr7CKfioEwwq9v4on3obizzvQahiENL5vujYuhWdP9lgV/EuHBDZh8t2lUNZnz9om+yDcMMQ7Kqwx6/ye8EtYCt8F/LF5o5ALPfYA/uVK/FbzF785lyIldd8cezqcxgxuhdUirah7WVf9LkcjLM89uKGRVdJVXEoBO7ege5yAbBSMRCznu5ARd4OrPTzwSMiahjxKRgDlpXh/Ru7wOpbDWiPVcZ/lBRhe1AKGg2E4dxb1fBiYjC6yvjh21mBWGxah+PnrMRtU/Oot9gjkjA0htwbZ4p5Z+YitbjILi89iT9X7YNS7itZfU8UjqeGE1TxxgOxMTgY6IY+MXxUCUqDiZKnaYFbLh1VGAsRCXn42jkeNsQkoL/PQ1YoXBw/VkphyTxV8uytDI5EjCMzwr5x3rl+bEbQYkxct4euNxaQ13QBZ2pkh2pPnpD+M+dwktgBdDOLA2WvG1zZtsf02KRx3Aw3gqyzAYqfLgUSnwINPSfJYlEhuiaqi75aFc4aXWnC35Ga0Cq+FXTVbGH9cidYzSyh1nvT8H5dGRicnEXqYivQmy8E8pEpaLpqIkrFzUPl6Rsh+U04Ks71RNywBro6K1nt65dA4KRLrsWFg8zWBgw5dgpEp1TAftSEAX8v+LVYEqqHlcn4oy5wzT8Xty2biVl+FXh601LytnMPaXxRRmVTlHD52RPAWOxD6ZMpsOuvntTJDBCBxjGauyEBi5SnAG51hAnK9vDs6AOiirepdk8etbI/TiMSPOGzWCJZ08XhonBKWvOTcUJkLlx4HIOlIj/Ii9rVeLFhJfssuQAWmo3D2LxbIJl7gFwrbIGaphT4vnYfST5XToXvD1Mr4w3wpOU9NTNIxbHav+m71fmsTHA4ebtMnBbbRKC6XB62bXPHWz7JaKWzDvc8yMQJURowOr2LuI+XAN8PZYIC3bW0bXkKsZ4RAcrCcTTFNw1HJUcpFNTQYAcNjHcRA6Hn0ri6nEUbpXBo1omALdsrgXf6PVm6RQhqht/RJYYREDdyAp/81aux3UEgG5aEVzZshOXxGXCyyBpfiCdARdlJdFrkTh+M/0j2f3VBa3d9Ijm3Fs6056Hr8mc0qF8fWjbfIm6/I6hQVx0EtM1G9Y3R+DzMBJxnxrEXJt8nHjXN7KzqY2icUkQU/hGnpacHyPv476T34XfyaMpjsj2jGgct3GGLViBdF5qNE11/kr2cEiaiF5nSFophDyowSq6LLi51wdJkC3ymWmbKvzSHHBUxpBE3qlA+4BV9Ua+E54rPkuo7CvByUg2tOZgFys6EvOmpwzByA00rAvFL6j7yRryRFu9hgTdxIr02bT3O8cygoZ5H6MHFTRzvfgHU73lFHE208aznLWJHKqE40hSeMC/J3CUM/DjPh/bpeozMFi8w3n4Yfp1/T4bcZ8OR2cvgffNjMsXIFCeKTqbLxYLwnnEYftQxIA/iX1Ezqko8Wgqx3FcZ9x2IpAMqC0l5VQCO83CkJsc2QaNsLGz3bYGKj3LYWbwao9qqabirAT5vsySd5rtAL/syK3VbEfO666jc4XPYcjoEVQ/FIGVEwDPZCQ3TKcQu7CfjGlRohl4TPtLSQechXVw8YzOYeVNy4Xw7/Rl+i+SY3OIylCPxzKTN+LBWCF9p2cCCnk52j0Mm56t8nc1xmY0hG/rIWoPXtO+aPdgeFMbUpa/JzsrpsOmvr03c/Iq+vGtLa6qCybhHNrBeqwo3PjiCIBwCgf4caJ1OE6y5v5ZMvfWWJnVk4o1XMvDmn+OYo9MBZ9eUk4sWSvBO/yKnpGcNNVsDYGa2Je7WOIgP4nZxIZVvyDzReqK66yh5q5DPvrn0mZQKy9PBwSPcxjEboWBaITby3TCx5iS1/HEVBrdy2F6xC3qkXtJ7Rqtp9Ohmcj1vlAbr5wtOYhIG67+gKX98sERvA+ytWQ/P7lyg2xcM0h63YviWr4cKTzhurqUaOSKvimG/rSGY0UVV1zf0qEUXrfRm2cgLxmj8YSZAkQbZb7ISTyktpgViDjA8/wNaj1jA3dPzUEoQDFfvKGBr9QS0blzItRh5gavEUYxX7yXe2lvpHG4tfHYdh7kHFsPPHaJ44EoHwbRYcNifRCpyVOBuSxGE7pgCKhe3AxnnDKUS5/k7r8ai6yFbHPGvgzuT2nC5aga8d51GnKOssXdONzt0cTkGPO1kKv0ekzm5rmBVEghX9iKJ+nCdPNfOIzEHesnRFcdwdeIfuuxGMllb60bsnQX055QzVHJKJr7V9ocsKUNIepAAdua1+K4wBKfunYqHrDsg53kkNhUn4cr6JpxhY8oqvV0AZ5zraFVwE2v3eSaun8gncTVIzqmbYz3VJHUG1nh//hFIcS4golhCPaK+cDGRVfCxQQUXe/FJZZQoF/doPienbkwKfzqx/no7yaSVvrDtci1nkdFAOrQy6Q39E0RJfQJ5evMsvdS2Cd5sTKZL9yrCij2PiZhLAAq2m9HSLHMsmC2KDovLsWP3V0IWz6QmG2aQbMEKMm/qbDJ4YhWo/aij6iYiWKC3BY5MdKW937/TzqEvZHKIGez0Eibdz5QxbHMZoGq34NPgJ0HXuaP44I0QnNl+CladasKM7Nl4vEMIrGcr49OMrdjpcQyP7HxMdzhlYElfM+58LQ9RWsdJltx4QYRPEdqvSwfVVzF0c9Fq5m1UCn7qDYCD3rXY7dBIpa6mUL22TJSXaINxMfGstb0m5ok+IU+lWHg/WEKf3loOrV/D4XR8PLk5IQOaZNOxT6GdWEmIE8vtidiu0szdbFEAEhVDonvMsbTyAeb1NrLJs5Lgk6E6J9i3idu0SpN7bTbCGN1MQ81vKrDtwwD3rbtbUFuhg2uM3ei6mmvcmJsV3O70U5ydVwJ8eGZoel8SSXbsK67TpJHbyZ1mnY64o6v4W/Ll8336/sYltsq6Co1kGrnRagf4aZzBttc24lk3R7LU5iJ8Kp1HepDA2nNb+Ip1lVigMY+qn9EWbHjJp0s2eaF1tTluup8B3et6OFtVLdry6ya76nYvjKTe4acGPqC+LxaRkJDpqHdhWCChso9N3SKKS2OlOb0wNwje2g59cYZ4u0VeEDj/OiYMJrBtn47BqeSLxGRZBwQqeJLZ4sfJqGgsdfRogNykK4z+3Zs0fNpxVlXoEtqOmEHi9Sa2P6CEs5rkjZ11Npgu9kQwTUGTu2Lwk/QnGcMi9Tds5vUY+l52I653CCahzEkupoUPVSejqcckXbr97EJ6v+wsoP50mjJmP1RxmWxESQIcPdnERscH0qBnBdgeZ47RPXbsIo2puGy1Excx4zA8buZD0/AiEBFoQSpbxyZblRDmoDKN7ZvM/noZz+nvHMtsiMkmEs3SnHMDDynPik6umcXxPXTRR243FzhuGewREqHSlxfjCmVTdpXJI44Zb845m+iCrvM3tlNMBt8t0IN5rv3kcXYWPP92FKxDr0LH1+tQIP6KXbzyCXupNhyt7At5efNU2TkRLBQ2++Af1xSutGcWqS+JAsfdnjjdeQlbtqpe4OJ/Bv2FvanohwXsrblCuPaEM9fgSVnzjQ/YCtvr7D7nBm7VtKXE/UMhd/HDY/JOVovj5hwGAW7HuFAX3PGhjPaGppAZOfHs3o09OHgrAwOC/9B5AcshbKCEDt6rpG/nj2MGNaSINk2Hc9pqRKTjMVQXeHNua4Sxw5+jfnFpfL+kK9Q2Uo3o2YmBk/tvGm3vBZO9b7PHJSLBe/xlwdvGauLtuRha99ni/BRCM0tSoeeHgFY/kcdVTZ24oLAatpRIEZGUQ5g60QBD3yI3NWcaa1O0HvPvm+LeC/J8MbMmrqDsIRVeJIW7lx5jtaaGQfVvb3iy9wwz98BhTlnMCo5VeqJZjzo4bZUh40Uj8adnHlfkuQOdyxRJpq8zTJVoZ+UMpoH+GEPcqqiMBpnjm3ca2eI5QQ6b/b2OeK9cSiT2b8eSZc5w+8NHbsMNK7hodI6Vi9MijZpyeO9SK+8ho8OskHDgivYuhoxp0bjl1W3ulCYDCTbxNGUwAGrJZJrXoYT1HyTYwbXiVGuGIjfKnMS7RnswZfNM1HNJhjknS6HnSh01ztcCzXXzOH8nHVxWvoIcej7MuceG0wWTrjG361qZUz7rqUV+F5rvrOGyxIpY0f6pYCbqiOVnMwVaQrY0KqMcRu8TonEwlJMK8ID0JnF4/M9B4PV0w82X8mSaoguRlNcE80AQiPmpoLpYDhv4RYm4/HlJwioMISxRDY47eGBr40RiYBSC542OwmtPFXpXX5s255cjfn2Hm/vHsq3Nk4iR6R7u7vqp7HdNKxI1YzxRyZtLz7vnY47Le0GPpj6tje1hZWcZ0eYFJdD9cA7mLBYlxc+aaNGlQ+xiqR0oXdiOztuzyDNXESJF+on85Bs0bp87Zm17ROyqe+m8UVv2FeNKXoyeQZnjFk1/XjzlZxg0opekEH5/O505sC2QNX4aQ81cNnNR7+PZ3A0CVF2bxOWaqTIO0T4krMsN7Vty4OJSZ4yeewZVg5VAosWU5HueAeneJtbZiaOqc+uZqQdXQsihA3hy2nSqeEOR+yHlBjbNtzmzWxPR5YsE++d0LLqXXwBJY2nyy9wJYrYGcF92byQG8wBTHdZzGy0rcSRZg7HctgLuOJjQEzek6D8SlrCu7jgXGRIHLmsnk18lC+nUa7NZ1Xox+J4fwSlHjaUf79ph7tH7xMJwESt+UolN0S3BgZ8ENXLf8hxGG1iN8dqs7bA5ev2Sgn0hKQK1KYOC6LmmMNGhoGkg4TJXffEYr9/+HjRfqsPBT585qfRykP5nKn20uYz7c6yKM8hwZBZIn8J5W1uxXOU3qdmSSca3W7P3v2vCtQsaglM27yjrf4J7N38O/JpiizMdOfgmrs2vPRQKDRdTmrbFnCYeF9qoA01HlBkDR8+Wwp7PhnRngzO7LcSBy8yKoj46XwRrNZWp+N6zrO/yTLRrGRV8farNubnsw6qem3xLpgDHvLJDh3tX0OG/wBbnzzU6/u+wxZL/CVuo/CdsofJ/D1s4B+7+L6jFXJP/X9RCQ3JArB4+ba6HGS1qyGd+cWOqAmhQ/QTEzlp6bmwdb46CNz5+JElXH0pkhPiu3GD8a7ZzWJGVT1QgimQCW7EkgfeoTZmt/gSCVLfnjG+YKO/BpvnMdqle/oz843z7kTzmer4DbNCYA7PylVFUag26FUfgiWAT7KgPph7ePlDs7E/UbHeAXE417LDjWM/8v6Hnt5fcudFc6lbbTyL/XKBJrscwVPgTfTsgDefCv9PEY/NQSamX9bKURaUmc0xfcw3CTPNoYq0HPlmZD1erf/9VlWryLayMvZJth3cTlWH7j3ou8VQyG9LgwhXuOcek9E5iD/UHc9dqo1jHbeKc54MuZkv4bcFdXwNm17RzzAlvytOYaMXMPjWRUzOdykif4cH5ovNk3t80/fzIAihlf5DhJ7LwaEwb+bkgDdb0niU+JZ/IwIEQOBYnCaJJSbTYNBkyDi2BbW+7WJWLheSTmR+4roiCowp25MJtFudM7iEmu50hSWsteai1idtYuwz8dF5y7ovNsN9xJ+4ZNSdVfbqEqr9j1xedgG1z7MAv1pOeiSmjqTtvcN9500GtbgVWXajAtetn4t7wDUTrK0e2i0aSodpvAvmDlqgQEkyDasPhjV4sFIS8I9qfLSB/sRqs/DwfTn4yxUd3HHBD8VI4s/8gq+e7Dws265D0NC0471KF13+9I7ptMzDTRgbFN0QS3dIG4uxnSxWLwiFpy170+LGcPkpg0fZhGMkb1Cd7z8uAZNoIdetcDCPdE2D3wzj2xfsDKBSUgF2t2axTOIO5nyRpQsV0yDx7DlZvIfTXgDQqhKnRNxV+0Gz9mm2THWQMChLxtLMfqZy1Gnp3/eb6JutAht1a8mB4gD79k8MrmGSCIXNt4VDwca7gZh2kjJOHMS636YwNUmjfEANFnpqQs0oYdCQauWcO0VTE9gJ94bAHhMPjwFX7KvHfOo5dM+wpyOmXoxpfOTrf/g5VzJoEQzezQPvQb6J8tppdEsVjDk+1Ap5DKj5UG0+NQk7S1MJ50LEljZQ6zMe5d+vZ66/i8cXn7XDhwEaaeuogZBXmEd0J6URh8yky50YMKL3KwBGrJOr3WRhfeDyj7bqTIbLpOxczwRymeBXRmy8k4GfpKtAmzphz3RVUXz4n/rumo19WFq5SSCWDj2dB65wbNC1SiD1OjuGlob147esXuvuqBAze4GEuq47lry6QxUsT2flv45HIDXEBNStRIm8LG3fHEDOlEtDPx4y6xdjB9ZibpFyQATm4j8ndIMof7FsBdy1OgIZnIXPthQx1jjHmUkROMB/SyuHw4SoI9pdkZR3O0Ym67fy94/L51xa95zYxr5n3Xs0g8yUc0v002aQgObzu0s/v11vPmCuvBMdDj6j/gzS47s38TbVVScgkLyzzy4d6ZUpj+iRpSbw4uySvlpTp1WDq22TwmyyNB1LO0h/Zx9nIK8OCOKkTrGqVCESv4qPWGmNwDpyABeISONfekOwYG8/zCK0FH+EIEtlTid2R6dRImUfcZwhhrG8QuCxfDd7D9uTegkLGMvUo25o7hgTuVaNTv26lGzZs5rz0tbmbHWpM4Qc1XpTjRUHC8ARO//U73rcBXfbh8m98/6r6hUWeonxRugJkpMzwhJAY/l62GifHJkJYqCEc+d5MNX7Yguk2BVL0fRdKxhdjTlsTqxO1Bk5GtLKfRhdQj5+aUKIYTWamHoZG3wVEPU8XDUkyrabmaOD5htHw/HsmDTtx/jNr+LDAE/T2m4HQK09aI3uJKt3Mh2+thsTEI5+u1p2Hpt3CuHinBu7JcIXAS3J45NUqeu13Jl6d9464Zd8kt+44YtUBEZS9KkoHDC2w4/wfZprYLhQ7SGDk/iFySqQLdj9aC5IJ0uDCpGFA8gQc+viCZL5zp0PvO6lQ6GnS9PEz26/awnFuIgvTiq5yQ4YNTIPETd7o5UzB60k23PSOa0zO8EfeijpOIKexmzdGUMpczukTnE9WZopEHvBEs+7CKt3zuHu3HP0W/Y3ofq2jt7ImkDT3z1T4ayGhLqM0eHefwGbZlb92OspcTmS5fXOSBUsbRBmH99NZS/csgbSJiMDKcAFTeiKAlyD8irlnd0HwUDqJ/zFtLf/F2gZmrPBS0FyxG/H2HHByTKae7GcywysbL+qWMS53z9LG4cW456kkJAqZQWecGs71XYjSkxbSf3yyMGSJCoh0SeGpeifouz9CXrq/IIZneLieShHfhSvQJWYLvN0dT0ItzEH9WjqxyYvApQfH4uXUJFp9WwqPfvSDTzQYNCNMiLTxQuKmrwvtSmagy6jiqPIpPMR00z2K95rMR7K42ENddIxRIM68ZIiz9sWzDeuLYdRHBSrXNjOdCgOCtCW++KCqmCaV6MPSsWPReUsuyurqQ8LBZ3Qo7Tc9rTFCI1tOcs0/kwUb+jiOqXRj7ii3szWlOTxD2TMsflVjWtQG+ZNX5wtOzAzlPVVxYy9LGfBsHz9kHCImC4zap7Aba21BnBVFn6QT+HOvNNbLMmScpg9UG/rA++Q4Sr58pDOmVrFbz+uw1QM8qlF0hkso49gBkQi2m/+TZ2lis3BdRibfRXM2F3FfjB0tX84su6HFVQ3k8k02R/NHj33nVe9eBuWh0bCuexbpOqmOV75koGhbMTF+zeKRABMq16aFE7rC2PTMO1T5+0b0X0KplIYY7FQIoOb+jfDyr198+WsV8/SKFQgJPtE9s2QgblkKud96g9QtHyGbtyzEd+gIE5O24I/XS8m5YRc49qAG5d9VUrvHC7EyOJf88u7lhrdvx0vzJ2JAO4DRyGs68bshah9ewknOSscpe9eTzUvaaaCmK7TtFIbe/VLwKPINjQ9IZbW+fKOnYquwe+UJsPBPwLi3ZeSyXyTK3gkCv6s36C2Ni9QhtoGeOjURk1bELOQmPGXde43YjxuOcAsHSpmJ3RncaftR9pPMWHbNASHBtBTC+Dj+ZIrXFggea89seq6bzqspH8PO3X2YG7hqA/pUwC7JHiGVRUnoV+NG74pTdob1VTrh136UKYmH1meFpGe2Dz1kUyVw37EKNxzWJwv6cyC0MR7FntbR520V5DbWCBxv8rn6gxHU1DQcjFymEOGWYxgb4Ql3C53g0enn1OltAl6YcpXe/Z5F2qzXwEe1WJgvZAOjOkWC0wtaOXUfBrVkM7Hw1CIcljqCl6W2sJHTxxKdpHxBTKAvyZkSDc9mJsHW9evI1YIocPhTysnHZ9IXyoUcLNoPniekoVn5AIYOjJL6aYuhkbcVD16cDV1xRmiSUs/suYasi94Bys/eQtRlgpjj5ebk5tESvvitBO7F7a88vhPlSgOVyBSpVMESkR5ulnCEwK8oSrCzo4Y9uyZJ0FVuDTvHyVA/uUR4ye+ni5rUacnWETqusZxNZo5gXHEqsOHlXIFTLus9QY0TyBzhMqd7cZbgTfRWCLisQ07nsyav53wcJjHiD/sExnlPBQE7prGfNbeysb+qBYf1ZXn2fcvhvtl7eq/EHH4PuWJfjynUvdHFCGFJMv9CKv4W7yGW0gxeOZoLMtn36ECUNn5eAnhhzQJwnC0l0Jk3SNSYTAx95AcT436xZd9ssHi1PdlfLQOv9dMEC9JD6abjlmD74A1ruOYK/QNJMN71IMH489zmfSLoaLQSb647CnfgElHL20Q5g3mczBWAux3Pqf/eSDxolwbnnj4kOfxKuo2Ycp9G77MtqRHMKoe98OabEZVTC8W5DTsBt5sDf+QhO4OE0GvTj+JNrx30JDnC9arNREl/UYj0y4WpzY3kww55ojm5jPseMwaoigs8MZyHRUwpHk66R1Mz7enAp242PrmclS5tI/udLNF5SBOPJJjAu0UpYD/015ckWXCOhtvwvf0y8Nx4nCOqp9nZH6UgaaQK0wzlYX5nGdetNIV4eZ/nHoh8Z0X9T4PypEwUqusm6vKHuA4xXfKyWI9GHDZiD0zTxqDUubhktALMspxwo8U+cvX3Mlj8cxraB4mijbQflLyIxv5BefDlt9Jl7mvxetBdMro0CfXZTOh6fpJVNFgNo9afuaLVI1RVfh81DtbF7u85cPfmLMoslMfp1+KpSJgr7ORKWOvwOOI9Zg7mbqqCkCeBxC5oOeh6uUPC/QGy6EQfKfzeQ1zzkuDc/Ikg1nGGbQ6cSY6FnWa7buRxEq9ciaauKZE1O8DejX7DBM6OYuUPhbL56osZ3xkOnLXLYX5fqxx7wbKUmV1rycXXroDrXnl0UGMWyHaGwC93CWyhLTQ8dSK343U2tHabg+uXWegj74iHn+jRWb2WmFwqDlNtjoFe01honq6OzyvC0eT+r7/35TzSbjgf+RFS+HqHKYxJt4LgDbuIk0UhjuvR47w/tNGPBXbwWvkhfbJpFSrdNkGLs6as0arjGPm6lt31JYN1CzrF9czVoMcLJpvmHzjPSd4vYdZsXMtzPrmAe7hHjNf+20Ow56sPR+rFuO0iK7nXYw8wuUNT2bQVS+DVphlwdXcdtanwBYnnCXDO5w49smQ87o2ZjK3NfGJonA0VxVtx6UWO+r6ZA4Y+QigqXkgnLr9M/dxnwe8tAVgY00gb5inCnxXqpGPBcjBo/8T9UMggM9O3oNuOJhhe84Jur6umFeq1uHS2GRyI/ETCOidS4d1nyDmJTGha7UZT3C9yGb+N2SWvutmMgQrWKKmV4Ws2cur8pazNFh024MVhQdLHw/yZmT5sjOh7ZovpNKa37QJf0feuoPXaCkixOY6vZSPIx0yWzP/UTCvKJFFH3Ag71+ewt9kirHYphim1dtSgaCO0eo9DJ6Gf3IQ+I1wQOwFNFfXA7rQBBCo4kAF1K9jz+BZ5nfOGE1+nD4ZBudS3KRrehFpAn6gzjJ0CwAy1k3k71CC7ORWmSeiSP6tl4JnOEL1kp0Dz26Oh6YolXLBTwNk1F0myhRNGuejiNttP9MmUCPpmhhvM3CtGa3KXYNNcT65x3xwUiY3HNVf7OGc9c9iT7IWdFTtIo8I8nHooB9bcQCriYYieP3Koj/V+ulRyKo7jTYOgmfOgvG2EWl5SgBDVcdRYNA9+FErhg8Zs2ltpjKrD48HH4gkpsOon59U0ydcbk0FzXgpKT3MAtUfVbMzBfEyKUiQi7Z28Mz66VNH+KkyuyqMd9y7wQvJCeS262lwLE0c6iwcxrl8YTme/EdD6i/wxKx4t7Dk1RlAy8yz7uqqOvrn/DOY/1kAfN29W8rgT72bYcf6EVntwqG4iAbF5pDf6KEyZYAZz360kWT3T8X3xNBhaFwtMWBIMTxoD6yYEUFPjGaj9haDa8wVwDBxh1qkMGBvLgNDGYlrtmEHrX6aRITcvtAh6Rs1LJf/+9zVy/N4FclmtSdB2KBneYTboaURjX6ktNp4xwma98SRUJQ02O8yir6oItyIklQ0SK+a0jmhQEeM+9p5kFS/Zy427550oOETSTTFsf1Psw9N8yWhRTnbJABNwNpYfs+u0QDF0Jeh3TsWuxEh8JIckNuQKtfonEfyDHKAtTgfrp3lS2LdAkCriDgFn6rBhKJ2Wd9hDZuZPctDzKO2eL0WKZV7zDqxPRe7KYUjaX0HbxabgngFtFM9Uh/b0bI7aWIDfXlloHNyI4yRlofmWDMTyN+ONjI/kselmlJLK5At1HSdVxdmYoRYClrNnEh/FXTAsI8tK6pbTpT9XCH61KOGe59Xw0HqIhh/Ww4NO76nQXHlO2kILICiRmuXawg3nOLghdIPO3V9Lqzpn4r6DDljwfJiWnIgkjNpOtPQlGLBfA6yab1PnbRbQDT/ZLicvGFO5B74HRMDaVgPInFzGavD2w/r8HfzYnjw6s0+CnDV3x7f5F8h03lJ4pxCEby6aYJftWjp3EiV37PPBPlmWPSQVRwp+26GC43RIjpDA7bn6f211LuxLlaWXk4qg3+sh6Yi+RmdoheDNu5QIf2mhXzeZ4YFFlVxj73oMGNTBSZrdROm+FdQYptIp8w3YS1VH4dAxFdT9ncL30DXD+SU32A29qaB+JQSv5Sdx/cqxpCSI4/SUj6D29H9Yc804yHRLxnv7Ozjjoa/E5JYiq+sxE/xsNenths34qfsmGzLZDLQirWFH20320dlokE+Xx1XbRCk5t4B4394NA12J2FiQSd6X17IXPeLoko4kMuqkA4dnXsGQp4bw87o0Hck15xq6rEiKIkfPv4kiE0r72NBbTXhhWiikqVfRVV+c2aEzHmAddZnoPB2LSVMjMf9rAPo/fkHPTO2hPfsnQU59Aukw1CV3iBKJXpUnyLRuZSupJau6Kph7XGLDsqZinLi7EXdV8ydvFhPDOIukC6ajQDBkOlEAjQ6CpWfGcwk4HntHjdiite3oNWqI0ZcOQKZ7Bqw7PY/+Dr9Fmo9loMy6dnbbU2RWdg6wvTqPOZ/Lp7nvaS5UdexK1ufJHDbpZzTbxHFs8NnLbNu1fE68yJy+v6VI5dq7TUVPnOXNXeII41TzQKvCiTiXubNNtkpkW9pHcvz0Qsw16mCyhstxQc1ZtB9yZZvaQ7GOGaLL+9Zw96+o4wez9fT+6GF6L0AbS4KKuJ2Pjf/GHT9Jpe43gdPYFRD/gOP0v5WA+OQi8HysimOkL8D6Bw/JP4WDtHL2ccyOEQZrNw12844Y7nm0BsNr2cOf9E2FpuSK8T8cu8LoDm7i0n+KcMc67wjGnf8qaNYDvuWJtYLkyjaBVPQN5vJrRcZ5awNjP07AKwlZDm+HLEEpIggD9Qu4bUbidNWHKpQ8Og3CmEnk0a5xOFzUSK16wknEBwH8DknkLx6dTh9WJ4LKQmHiL9fABtUvBKuMNPrQZxNOkZPHuAQ+q6VzArfAQbo0dQmJC6yBv4YBaUG7aO7WYxA4qwjGZ52glsNiOCq3hr2X/pgcuTGPRKpM5c60stwzEzM284IS3frCV5Dzq4aTLulh/CrVBOff7RZo9PJ46s25fNWKSMZK21Ow0/E8r2X9Co4f1E9W7VfBX8JetJrZRB423KL739whcQ5x1Dxanc7WF8U0fEfG3PhI3vJOkpA6cYhO7OKmXFUAsz/L8NL1OBjQekJBuo+tnOCGybYP2VdJhWTfx9XkxPBGCAkkJDTnObl0VZ9KbswlBdKF7PuNSaTzjzzBPRtgMM4GTI7UU7vxUwRrf0+htmeyac6uNbDYIhq+J46yAf4cE93YKgjvv8gN+e6kF/VLaGi7KrgvjQXOvoqEZGaxYm0a/G80Bc7uTYGP6A6L2gZI0Pk5JPOcFfB/JFNR4Tp2tp49zsmK5F5Oncn2HKrkZTt2cm/YW4LNNrKUpLtzTX2xTO2TfsG+l2O4o3PVudaFyU0J8u8Exc9yTFVtknmbIs4zwTGekBaWgQ0ND+n7Of1kk507BuxeD8n9sjisU03lbf3QN2GQTPiiCcEMRwqv/40P7urTO1bGsFV8O1K12aA1/gq95PWJ9fD8mxOpr+TOi8QQU54p+85KB2Z7GbNSqidBQlGPGHgkgeo4YZSduYmIjlkGfc3FQKVz6NrpR0mW41TOdXwAZ22oQUb4acTL7gHXtjKTHTlVzg+9Zs1vCt3Eiir3CYbyj7EizjFsfLyrQPZ6CdsUJ8HtzY0QbCqwh/ziRIje3UN7/8ZrOXMIZHs749ELY1BrajwNmLQP22VM0fuQHgoWPKVCT2whWUmRpuv4wHSbaMxOisGUsdPh3vVOzvHqQWB71Zq21JSRweMK5Ga0L+Tp5pHXg5bQqXePbtwcxaJBEnYWesD8aR7cgUpLzJD+wD2N2A3HZp7AS9N7WM+hbuLp8Y1zOBkDrV4iZLzVDJTYdQoEw6q0cNZsXOMWxDa2zILGLQakf6UFHh6s5ySbFcihLDt4uTsI5BhXMnqeBxu/F8OuvovEWXYOrmjPJAeyquiibAX4FD8NRjzNcHLEKDV3moCHL6nQsWnluOHEFJxUlkI2X1yEm+/IY8KtYdIe/5r+UrIgt7TUcZNJGjosFRYxEP2v/ab/tpT+O8cK/S8KF/73+d+7T/9PkX9rPf+Wc/6d/1Wk8e9HlvyHyFixf1kif4fP39VP79/dfwPcRU3i"


NC_ = 8
NODES, CH, DIN, DOUT = 50000, 64, 16, 25
TOK = NODES * CH
TPC = TOK // NC_
NSTREAM = 2
TPS = TPC // NSTREAM


def _factors():
    d = np.load(io.BytesIO(zlib.decompress(base64.b64decode(_WBLOB))))
    return d["V"], d["C"]   # (16,64) f32, (25,64) f32


def _np_compute(t1, t2, V, C):
    x = t1.reshape(-1, DIN)
    y = t2.reshape(-1, DIN)
    out = np.empty((TOK, DOUT), dtype=np.float32)
    for i in range(0, TOK, 262144):
        sl = slice(i, min(i + 262144, TOK))
        pq = (x[sl] @ V) * (y[sl] @ V)
        out[sl] = pq @ C.T
    return out.reshape(NODES, CH, DOUT)


# ----------------------------- bass path ------------------------------------

FD = 512                    # matmul free size (one psum bank)
PTOK = 2 * FD               # tokens/stream per pair
NPAIR = -(-TPS // PTOK)     # 196
TPS_PAD = NPAIR * PTOK      # 200704
IGP = 7                     # pairs per input DMA group
OGP = 14                    # pairs per output DMA group
OC_DVE_EVERY = 2            # every 2nd outcopy goes to VectorE

_BASS_CACHE = {}


def _build_nc():
    import concourse.mybir as mybir
    from concourse import bacc
    from concourse.tile import TileContext

    dt = mybir.dt
    nc = bacc.Bacc(None, target_bir_lowering=False)
    u = nc.dram_tensor("u", [NSTREAM, 32, TPS_PAD], dt.bfloat16, kind="ExternalInput")
    w_sp = nc.dram_tensor("w_sp", [128, 64], dt.bfloat16, kind="ExternalInput")
    w_sq = nc.dram_tensor("w_sq", [128, 64], dt.bfloat16, kind="ExternalInput")
    w_ct = nc.dram_tensor("w_ct", [128, 32], dt.bfloat16, kind="ExternalInput")
    o = nc.dram_tensor("o", [4, DOUT, NPAIR * FD], dt.bfloat16, kind="ExternalOutput")

    with TileContext(nc) as tc:
        with (
            tc.tile_pool(name="wpool", bufs=1) as wpool,
            tc.tile_pool(name="upool", bufs=3) as upool,
            tc.tile_pool(name="qspool", bufs=8) as qspool,
            tc.tile_pool(name="pqpool", bufs=10) as pqpool,
            tc.tile_pool(name="obpool", bufs=3) as obpool,
            tc.tile_pool(name="ppsum", bufs=3, space="PSUM") as ppsum,
            tc.tile_pool(name="qpsum", bufs=3, space="PSUM") as qpsum,
            tc.tile_pool(name="opsum", bufs=2, space="PSUM") as opsum,
        ):
            sp_t = wpool.tile([128, 64], dt.bfloat16, tag="sp")
            sq_t = wpool.tile([128, 64], dt.bfloat16, tag="sq")
            ct_t = wpool.tile([128, 32], dt.bfloat16, tag="ct")
            nc.sync.dma_start(out=sp_t[:, :], in_=w_sp[:, :])
            nc.sync.dma_start(out=sq_t[:, :], in_=w_sq[:, :])
            nc.sync.dma_start(out=ct_t[:, :], in_=w_ct[:, :])

            u_t = None
            ob_t = None
            hist = []

            def emit_tail(item, idx):
                pq_, obt_, osl_ = item
                pq0, pq1 = pq_
                ob = opsum.tile([128, FD], dt.float32, tag="o")
                nc.tensor.matmul(
                    out=ob[64:96, :], lhsT=ct_t[0:64, :], rhs=pq0[0:64, :],
                    start=True, stop=True, tile_position=(0, 64))
                nc.tensor.matmul(
                    out=ob[96:128, :], lhsT=ct_t[0:64, :], rhs=pq1[0:64, :],
                    start=True, stop=True, tile_position=(0, 96))
                nc.tensor.matmul(
                    out=ob[32:64, :], lhsT=ct_t[64:128, :], rhs=pq0[64:128, :],
                    start=True, stop=True, tile_position=(64, 32))
                nc.tensor.matmul(
                    out=ob[0:32, :], lhsT=ct_t[64:128, :], rhs=pq1[64:128, :],
                    start=True, stop=True, tile_position=(64, 0))
                if idx % OC_DVE_EVERY == 0:
                    nc.vector.tensor_copy(obt_[:, osl_], ob[:, :])
                else:
                    nc.scalar.copy(out=obt_[:, osl_], in_=ob[:, :])
                if idx % OGP == OGP - 1:
                    og = idx // OGP
                    ogsl = slice(og * OGP * FD, (og + 1) * OGP * FD)
                    _oe = [nc.sync, nc.gpsimd]
                    for b in range(4):
                        _oe[b % 2].dma_start(
                            out=o[b, :, ogsl], in_=obt_[32 * b: 32 * b + DOUT, :])

            for pr in range(NPAIR):
                if pr % IGP == 0:
                    g = pr // IGP
                    gsl = slice(g * IGP * PTOK, (g + 1) * IGP * PTOK)
                    u_t = upool.tile([128, IGP * PTOK], dt.bfloat16, tag="u")
                    _ie = [nc.gpsimd, nc.sync]
                    _ie[0].dma_start(out=u_t[0:32, :], in_=u[0, :, gsl])
                    _ie[1].dma_start(out=u_t[64:96, :], in_=u[1, :, gsl])
                if pr % OGP == 0:
                    ob_t = obpool.tile([128, OGP * FD], dt.bfloat16, tag="ob")

                ip = pr % IGP
                pqs = []
                for h in range(2):
                    usl = slice(ip * PTOK + h * FD, ip * PTOK + (h + 1) * FD)
                    qb = qpsum.tile([128, FD], dt.float32, tag="q")
                    nc.tensor.matmul(
                        out=qb[0:64, :], lhsT=sq_t[0:32, :], rhs=u_t[0:32, usl],
                        start=True, stop=True, tile_position=(0, 0))
                    nc.tensor.matmul(
                        out=qb[64:128, :], lhsT=sq_t[64:96, :], rhs=u_t[64:96, usl],
                        start=True, stop=True, tile_position=(64, 64))
                    qs = qspool.tile([128, FD], dt.float32, tag="qs")
                    nc.scalar.copy(out=qs[:, :], in_=qb[:, :])
                    pb = ppsum.tile([128, FD], dt.float32, tag="p")
                    nc.tensor.matmul(
                        out=pb[0:64, :], lhsT=sp_t[0:32, :], rhs=u_t[0:32, usl],
                        start=True, stop=True, tile_position=(0, 0))
                    nc.tensor.matmul(
                        out=pb[64:128, :], lhsT=sp_t[64:96, :], rhs=u_t[64:96, usl],
                        start=True, stop=True, tile_position=(64, 64))
                    pq = pqpool.tile([128, FD], dt.bfloat16, tag="pq")
                    nc.vector.tensor_mul(pq[:, :], pb[:, :], qs[:, :])
                    pqs.append(pq)
                hist.append((pqs, ob_t, slice((pr % OGP) * FD, (pr % OGP + 1) * FD)))
                if len(hist) > 3:
                    emit_tail(hist.pop(0), pr - 3)
            for k, item in enumerate(hist):
                emit_tail(item, NPAIR - len(hist) + k)
    nc.finalize()
    return nc


def _make_weights(V, C, BF16):
    sp = np.zeros((128, 64), dtype=np.float32)
    sq = np.zeros((128, 64), dtype=np.float32)
    for blk in range(4):
        sp[32 * blk: 32 * blk + 16, :] = V
        sq[32 * blk + 16: 32 * blk + 32, :] = V
    ct = np.zeros((128, 32), dtype=np.float32)
    ct[0:64, 0:25] = C.T
    ct[64:128, 0:25] = C.T
    return sp.astype(BF16), sq.astype(BF16), ct.astype(BF16)


def _prep_inputs(t1, t2, BF16):
    xT = np.ascontiguousarray(t1.reshape(TOK, DIN).T.astype(BF16))
    yT = np.ascontiguousarray(t2.reshape(TOK, DIN).T.astype(BF16))
    xr = xT.reshape(DIN, NC_, NSTREAM, TPS)
    yr = yT.reshape(DIN, NC_, NSTREAM, TPS)
    ins = []
    for c in range(NC_):
        uc = np.zeros((NSTREAM, 32, TPS_PAD), dtype=BF16)
        uc[:, 0:16, :TPS] = xr[:, c].transpose(1, 0, 2)
        uc[:, 16:32, :TPS] = yr[:, c].transpose(1, 0, 2)
        ins.append(uc)
    return ins


def _postprocess(outs):
    full = np.empty((TOK, DOUT), dtype=np.float32)
    for c in range(NC_):
        oc = outs[c].reshape(4, DOUT, NPAIR, FD)
        for s, (bh0, bh1) in enumerate(((2, 3), (1, 0))):
            arr = np.stack((oc[bh0], oc[bh1]), axis=0).transpose(2, 0, 3, 1)
            flat = arr.reshape(TPS_PAD, DOUT)[:TPS].astype(np.float32)
            base = c * TPC + s * TPS
            full[base: base + TPS] = flat
    return full.reshape(NODES, CH, DOUT)


def _get_bass():
    if "state" not in _BASS_CACHE:
        import sys as _sys
        if "/opt/trn_rl_repo" not in _sys.path:
            _sys.path.insert(0, "/opt/trn_rl_repo")
        import ml_dtypes
        from concourse.bass_utils import run_bass_kernel_spmd
        BF16 = ml_dtypes.bfloat16
        V, C = _factors()
        sp, sq, ct = _make_weights(V, C, BF16)
        nc = _build_nc()
        _BASS_CACHE["state"] = (nc, sp, sq, ct, BF16, run_bass_kernel_spmd)
    return _BASS_CACHE["state"]


def kernel(tensor_1, tensor_2):
    t1 = np.asarray(tensor_1, dtype=np.float32)
    t2 = np.asarray(tensor_2, dtype=np.float32)
    try:
        nc, sp, sq, ct, BF16, run_spmd = _get_bass()
        u_list = _prep_inputs(t1, t2, BF16)
        in_maps = [
            {"u": u_list[c], "w_sp": sp, "w_sq": sq, "w_ct": ct}
            for c in range(NC_)
        ]
        res = run_spmd(nc, in_maps, core_ids=list(range(NC_)))
        return _postprocess([r["o"] for r in res.results])
    except Exception:
        V, C = _factors()
        return _np_compute(t1, t2, V, C)


if __name__ == "__main__":
    rng = np.random.default_rng(0)
    a = rng.standard_normal((NODES, CH, DIN), dtype=np.float32)
    b = rng.standard_normal((NODES, CH, DIN), dtype=np.float32)
    o = kernel(a, b)
    print(o.shape, o.dtype)
